# revision 10
# baseline (speedup 1.0000x reference)
"""Epipolar attention kernel for Trainium2 (8 NeuronCores, batch-parallel).

Host does O(B*3^3) geometry + O(N)/O(N*C) input prep (coefficient splits,
fp8 quantization of f_src, colsum, Lipschitz row-max bound); the device does
all O(N^2) / O(N^2*C) work:

  stage1 (i-rows on partitions):
      d[i,j]  = A_j x_i + B_j y_i + C_j        (PE, exact triple-bf16 split)
      xa      = |d|                            (DVE bitcast sign-mask)
      rhat_i  = sum_j exp(xa - Mhat_i)         (ACT, per-partition bias + accum)
  beta_i = Mhat_i + ln rhat_i                  (DVE exponent-field ln, batched
      4 stripes at a time so it overlaps the stage-1 ACT tail)
      broadcast beta across partitions         (PE transpose + one-hot matmuls)
  stage2 (j-rows on partitions):
      xT      = |dT| - beta_i                  (DVE subtract; abs interleaved)
      u       = exp(xT)                        (ACT)          == e/r, exact softmax
      E2T     = exp(-u), S_j = sum_i E2T       (ACT + accum)
      W8      = E2T*(2^14/S_j) - 16            (GpSimd -> fp8e4) [attn = 1/N + W/2^14]
  stage3 (c-blocks on partitions, outT = fs^T @ W^T):
      psum[c,i] = sum_j fs8[j,c] * W8[j,i]     (PE, fp8 DoubleRow, K=256/mm)
      outT = f16(psum + 2^14*F_c/N)            (ACT Identity / DVE alternating)
Host: out = outT.T * 2^-14. The double softmax identity:
  softmax_i(1 - softmax_j(5(d-thre))) == E2/colsum(E2).
"""

import numpy as np
import ml_dtypes

import concourse.bass as bass
import concourse.bacc as bacc
import concourse.tile as tile
from concourse import mybir
from concourse.bass_utils import run_bass_kernel_spmd

B, C, H, W = 8, 1152, 32, 32
N = H * W           # 1024
P = 128
NT = N // P         # 8
CB = C // P         # 9
F32 = mybir.dt.float32
F16 = mybir.dt.float16
BF16 = mybir.dt.bfloat16
FP8 = mybir.dt.float8e4
I32 = mybir.dt.int32
BFNP = ml_dtypes.bfloat16
F8NP = ml_dtypes.float8_e4m3

SCALE = 16384.0     # 2^14 scaling of the centered attention W
LN2 = 0.6931471805599453

# log2(m) ~ c3*(m-r1)*(m^2 + pm*m + q) on [1,2]  (factored minimax cubic)
_LOG2_COEF = np.polyfit(np.cos(np.pi * (np.arange(64) + .5) / 64) * .5 + 1.5,
                        np.log2(np.cos(np.pi * (np.arange(64) + .5) / 64) * .5 + 1.5), 3)
_C3, _C2, _C1, _C0 = [float(v) for v in _LOG2_COEF]
_ROOTS = np.roots(_LOG2_COEF)
_R1 = float(_ROOTS[np.argmin(np.abs(_ROOTS.imag))].real)     # real root (~1.0)
_QUAD = np.real(np.poly([r for r in _ROOTS
                         if abs(r - _R1) > 1e-9]))            # m^2 + pm*m + q
_PM, _Q = float(_QUAD[1]), float(_QUAD[2])

TRACE = False
LAST_RESULTS = None


# ----------------------------------------------------------------- device ---

def _build_nc():
    nc = bacc.Bacc()
    abc9 = nc.dram_tensor("abc9", (9, N), BF16, kind="ExternalInput")
    xy9 = nc.dram_tensor("xy9", (9, N), BF16, kind="ExternalInput")
    smallc = nc.dram_tensor("smallc", (P, NT + NT + CB), F32,
                            kind="ExternalInput")  # [-Mhat | Mhat-127*ln2 | F*2^14/N]
    oneh = nc.dram_tensor("oneh", (NT, 4 * P), BF16, kind="ExternalInput")
    identD = nc.dram_tensor("ident", (P, P), BF16, kind="ExternalInput")
    fs8 = nc.dram_tensor("fs8", (N, C), FP8, kind="ExternalInput")
    outT = nc.dram_tensor("outT", (C, N), F16, kind="ExternalOutput")

    AF = mybir.ActivationFunctionType
    AO = mybir.AluOpType

    with tile.TileContext(nc) as tc:
        with (
            tc.tile_pool(name="consts", bufs=1) as consts,
            tc.tile_pool(name="persist", bufs=1) as persist,
            tc.tile_pool(name="pxa", bufs=3) as pxa,
            tc.tile_pool(name="pxT", bufs=3) as pxT,
            tc.tile_pool(name="pscr", bufs=1) as pscr,
            tc.tile_pool(name="pu", bufs=3) as pu,
            tc.tile_pool(name="pe2", bufs=6) as pe2,
            tc.tile_pool(name="posb", bufs=4) as posb,
            tc.tile_pool(name="stats", bufs=8) as stats,
            tc.tile_pool(name="P1", bufs=2, space="PSUM") as P1,
            tc.tile_pool(name="psC", bufs=4, space="PSUM") as psC,
        ):
            # ---- consts / inputs ----
            xy_sb = consts.tile([9, N], BF16, tag="xy")
            nc.sync.dma_start(out=xy_sb, in_=xy9[:, :])
            abc_sb = consts.tile([9, N], BF16, tag="abc")
            nc.scalar.dma_start(out=abc_sb[:, :512], in_=abc9[:, :512])
            nc.scalar.dma_start(out=abc_sb[:, 512:], in_=abc9[:, 512:])
            small_sb = consts.tile([P, NT + NT + CB], F32, tag="smallc")
            nc.sync.dma_start(out=small_sb, in_=smallc[:, :])
            mneg_sb = small_sb[:, 0:NT]
            mofs_sb = small_sb[:, NT:2 * NT]
            f14_sb = small_sb[:, 2 * NT:2 * NT + CB]
            oneh_sb = consts.tile([NT, 4 * P], BF16, tag="oneh")
            nc.scalar.dma_start(out=oneh_sb, in_=oneh[:, :])
            ident = consts.tile([P, P], BF16, tag="ident")
            nc.scalar.dma_start(out=ident, in_=identD[:, :])

            fs8_sb = persist.tile([P, NT, C], FP8, tag="fs8")
            for s in range(NT):
                nc.sync.dma_start(out=fs8_sb[:, s, :], in_=fs8[s * P:(s + 1) * P, :])

            W8_sb = persist.tile([P, NT, N], FP8, tag="W8")
            dabs_all = persist.tile([P, NT, N], F32, tag="dabs")
            Bm_sb = persist.tile([P, N], F32, tag="Bm")
            bTa_sb = persist.tile([NT, P], BF16, tag="bTa")
            bTb_sb = persist.tile([NT, P], BF16, tag="bTb")
            bc3a = persist.tile([P, NT], BF16, tag="bc3a")
            bc3b = persist.tile([P, NT], BF16, tag="bc3b")
            rcol = persist.tile([P, NT], F32, tag="rcol")
            bcola = persist.tile([P, 4], F32, tag="bcola")
            bcolb = persist.tile([P, 4], F32, tag="bcolb")

            # preload the Exp ACT table before inputs land
            dummy = stats.tile([1, 2], F32, tag="dummy")
            nc.gpsimd.memset(dummy, 0.0)
            dummy2 = stats.tile([1, 2], F32, tag="dummy2")
            nc.scalar.activation(out=dummy2, in_=dummy, func=AF.Exp)

            # ---- stage 1: i-oriented pass -> rhat ----
            for it in range(NT):
                psd = P1.tile([P, N], F32, tag="psd")
                for h in range(2):
                    nc.tensor.matmul(
                        psd[:, h * 512:(h + 1) * 512],
                        lhsT=xy_sb[:, it * P:(it + 1) * P],
                        rhs=abc_sb[:, h * 512:(h + 1) * 512],
                        start=True, stop=True,
                    )
                xa = pxa.tile([P, N], F32, tag="xa")
                nc.vector.tensor_scalar(
                    out=xa.bitcast(I32), in0=psd.bitcast(I32),
                    scalar1=0x7FFFFFFF, scalar2=None, op0=AO.bitwise_and,
                )
                e_scr = pscr.tile([P, N], BF16, tag="escr")
                nc.scalar.activation(
                    out=e_scr, in_=xa, func=AF.Exp, bias=mneg_sb[:, it:it + 1],
                    accum_out=rcol[:, it:it + 1],
                )

            # ---- beta = Mhat + ln(rhat), exponent-field log2; two batches of
            # 4 stripes so batch A overlaps the stage-1 ACT tail ----
            def emit_beta(sl, bcol):
                rI = rcol[:, sl].bitcast(I32)
                expI = stats.tile([P, 4], I32, tag="expI", name=f"expI{sl.start}")
                nc.vector.tensor_scalar(
                    out=expI, in0=rI, scalar1=23, scalar2=None,
                    op0=AO.logical_shift_right,
                )
                Ef = stats.tile([P, 4], F32, tag="Ef", name=f"Ef{sl.start}")
                nc.vector.tensor_copy(Ef, expI)     # int -> float convert
                mant = stats.tile([P, 4], F32, tag="mant", name=f"mant{sl.start}")
                nc.vector.tensor_scalar(
                    out=mant.bitcast(I32), in0=rI, scalar1=0x007FFFFF,
                    scalar2=0x3F800000, op0=AO.bitwise_and, op1=AO.bitwise_or,
                )
                t2 = stats.tile([P, 4], F32, tag="t2", name=f"t2{sl.start}")
                nc.vector.scalar_tensor_tensor(
                    out=t2, in0=mant, scalar=_PM, in1=mant, op0=AO.add, op1=AO.mult)
                nc.vector.tensor_scalar(out=t2, in0=t2, scalar1=_Q, scalar2=None,
                                        op0=AO.add)
                t3 = stats.tile([P, 4], F32, tag="t3", name=f"t3{sl.start}")
                nc.vector.scalar_tensor_tensor(
                    out=t3, in0=mant, scalar=_R1, in1=t2, op0=AO.subtract,
                    op1=AO.mult)
                t4 = stats.tile([P, 4], F32, tag="t4", name=f"t4{sl.start}")
                nc.vector.scalar_tensor_tensor(
                    out=t4, in0=t3, scalar=_C3, in1=Ef, op0=AO.mult, op1=AO.add)
                nc.vector.scalar_tensor_tensor(
                    out=bcol, in0=t4, scalar=LN2, in1=mofs_sb[:, sl],
                    op0=AO.mult, op1=AO.add)

            def emit_split(bcol, bc3):
                # bc3 = [hi(4) | lo(4)] bf16 split of bcol
                nc.vector.tensor_copy(bc3[:, 0:4], bcol)
                nc.vector.tensor_tensor(out=bc3[:, 4:8], in0=bcol,
                                        in1=bc3[:, 0:4], op=AO.subtract)

            emit_beta(slice(0, 4), bcola)
            emit_split(bcola, bc3a)
            emit_beta(slice(4, 8), bcolb)
            emit_split(bcolb, bc3b)

            # PE: transpose + one-hot broadcast per batch -> psB halves
            psTa = P1.tile([NT, P], BF16, tag="psd", name="psTa")
            nc.tensor.transpose(psTa, bc3a, ident)
            nc.vector.tensor_copy(bTa_sb, psTa)
            psBa = P1.tile([P, 512], F32, tag="psd", name="psBa")
            for loc in range(4):
                nc.tensor.matmul(
                    psBa[:, loc * P:(loc + 1) * P],
                    lhsT=oneh_sb[:, loc * P:(loc + 1) * P],
                    rhs=bTa_sb,
                    start=True, stop=True,
                )
            psTb = P1.tile([NT, P], BF16, tag="psd", name="psTb")
            nc.tensor.transpose(psTb, bc3b, ident)
            nc.vector.tensor_copy(bTb_sb, psTb)
            psBb = P1.tile([P, 512], F32, tag="psd", name="psBb")
            for loc in range(4):
                nc.tensor.matmul(
                    psBb[:, loc * P:(loc + 1) * P],
                    lhsT=oneh_sb[:, loc * P:(loc + 1) * P],
                    rhs=bTb_sb,
                    start=True, stop=True,
                )
            nc.scalar.activation(out=Bm_sb[:, 0:512], in_=psBa, func=AF.Copy)
            nc.scalar.activation(out=Bm_sb[:, 512:], in_=psBb, func=AF.Copy)

            # ---- stage 2a matmuls (emitted after broadcast so the PE queue
            # can't block it) + stage 2b j-oriented softmax chain -> W8 ----
            S1s = [None] * NT
            e2s = [None] * NT
            invs14 = [None] * NT

            def emit_dT(u):
                psd = P1.tile([P, N], F32, tag="psd", name=f"psd2_{u}")
                for h in range(2):
                    nc.tensor.matmul(
                        psd[:, h * 512:(h + 1) * 512],
                        lhsT=abc_sb[:, u * P:(u + 1) * P],
                        rhs=xy_sb[:, h * 512:(h + 1) * 512],
                        start=True, stop=True,
                    )
                return psd

            def emit_abs(u, psd, eng):
                if eng is nc.vector:
                    eng.tensor_scalar(
                        out=dabs_all[:, u, :].bitcast(I32), in0=psd.bitcast(I32),
                        scalar1=0x7FFFFFFF, scalar2=None, op0=AO.bitwise_and,
                    )
                else:
                    eng.scalar_tensor_tensor(
                        out=dabs_all[:, u, :], in0=psd, scalar=-1.0, in1=psd,
                        op0=AO.mult, op1=AO.max,
                    )

            def emit_exps(u, xT, half=None):
                # exp + exp(-u) with colsum accumulate
                sl = slice(None) if half is None else (
                    slice(0, 512) if half == 0 else slice(512, N))
                uT = pu.tile([P, N], BF16, tag="uT", name=f"uT{u}")
                nc.scalar.activation(out=uT[:, sl], in_=xT[:, sl], func=AF.Exp)
                if e2s[u] is None:
                    e2s[u] = pe2.tile([P, N], F16, tag="E2T", name=f"E2T{u}")
                tag = f"S1{u}" if half is None else f"S1{u}_{half}"
                S1 = stats.tile([P, 1], F32, tag="S1", name=tag)
                nc.scalar.activation(
                    out=e2s[u][:, sl], in_=uT[:, sl], func=AF.Exp, scale=-1.0,
                    accum_out=S1,
                )
                return S1

            def emit_inv(u, S1):
                S1s[u] = S1
                inv = stats.tile([P, 1], F32, tag="invS", name=f"invS{u}")
                nc.vector.reciprocal(inv, S1)
                invs14[u] = stats.tile([P, 1], F32, tag="invS14", name=f"iS14{u}")
                nc.vector.tensor_scalar_mul(invs14[u], inv, SCALE)

            def emit_w8(u, eng):
                eng.tensor_scalar(
                    out=W8_sb[:, u, :], in0=e2s[u], scalar1=invs14[u],
                    scalar2=SCALE / N, op0=AO.mult, op1=AO.subtract,
                )

            # PE emits all dT stripes up-front (cheap; paced by P1 rotation).
            # Stage-2b engine budget: DVE takes abs(0-5), all subtracts, the
            # stats, and W8(4-7); gpsimd takes W8(0-3) plus the left halves
            # of abs(6,7) (DVE does their right halves). recip/invS14 for
            # stripe u-1 is emitted after sub(u) so it never queues a stall
            # in front of the next subtract.
            psds = [emit_dT(u) for u in range(NT)]

            def emit_abs_half(u, psd, eng, half):
                # gpsimd can't read PSUM; the off-DVE half goes to ACT (Abs
                # is resident in the exp table set)
                sl = slice(0, 512) if half == 0 else slice(512, N)
                if eng is nc.scalar:
                    eng.activation(out=dabs_all[:, u, sl], in_=psd[:, sl],
                                   func=AF.Abs)
                else:
                    eng.tensor_scalar(
                        out=dabs_all[:, u, sl].bitcast(I32),
                        in0=psd[:, sl].bitcast(I32),
                        scalar1=0x7FFFFFFF, scalar2=None, op0=AO.bitwise_and,
                    )

            pend = [None] * NT   # S1 awaiting emit_inv
            xts = [None] * NT
            for u in range(NT):
                if u < 6:
                    emit_abs(u, psds[u], nc.vector)
                else:
                    emit_abs_half(u, psds[u], nc.scalar, 0)
                    emit_abs_half(u, psds[u], nc.vector, 1)
                xts[u] = pxT.tile([P, N], BF16, tag="xT", name=f"xT{u}")
                if u == 0:
                    # split halves so ACT can start on Bm-left early
                    nc.vector.tensor_tensor(
                        out=xts[0][:, :512], in0=dabs_all[:, 0, :512],
                        in1=Bm_sb[:, :512], op=AO.subtract)
                    s0l = emit_exps(0, xts[0], half=0)
                    nc.vector.tensor_tensor(
                        out=xts[0][:, 512:], in0=dabs_all[:, 0, 512:],
                        in1=Bm_sb[:, 512:], op=AO.subtract)
                    s0r = emit_exps(0, xts[0], half=1)
                    S1 = stats.tile([P, 1], F32, tag="S1", name="S10")
                    nc.vector.tensor_tensor(out=S1, in0=s0l, in1=s0r, op=AO.add)
                    pend[0] = S1
                else:
                    nc.vector.tensor_tensor(
                        out=xts[u], in0=dabs_all[:, u, :], in1=Bm_sb,
                        op=AO.subtract)
                    pend[u] = emit_exps(u, xts[u])
                if 1 <= u <= 5:
                    emit_inv(u - 1, pend[u - 1])
                    if u - 1 < 4:
                        emit_w8(u - 1, nc.gpsimd)
            # tail: emit stats just-in-time so nothing queues a long stall
            # ahead of a ready W8
            emit_inv(5, pend[5])
            emit_w8(4, nc.vector)
            emit_w8(5, nc.vector)
            emit_inv(6, pend[6])
            emit_w8(6, nc.vector)
            emit_inv(7, pend[7])
            emit_w8(7, nc.vector)

            # ---- stage 3: outT[c,i] = sum_j fs8[j,c]*W8[j,i] (+ F term) ----
            for cb in range(CB):
                for ic in range(2):
                    ps = psC.tile([P, 512], F32, tag="oc")
                    for s in range(4):
                        nc.tensor.matmul(
                            ps,
                            lhsT=fs8_sb[:, 2 * s:2 * s + 2, cb * P:(cb + 1) * P],
                            rhs=W8_sb[:, 2 * s:2 * s + 2, ic * 512:(ic + 1) * 512],
                            start=(s == 0), stop=(s == 3),
                            perf_mode=mybir.MatmulPerfMode.DoubleRow,
                        )
                    osb = posb.tile([P, 512], F16, tag="osb")
                    k = cb * 2 + ic
                    if k % 2 == 0:
                        nc.scalar.activation(
                            out=osb, in_=ps, func=AF.Identity,
                            bias=f14_sb[:, cb:cb + 1],
                        )
                    else:
                        nc.vector.tensor_scalar(
                            out=osb, in0=ps, scalar1=f14_sb[:, cb:cb + 1],
                            scalar2=None, op0=AO.add,
                        )
                    dma_eng = nc.sync if k % 2 == 0 else nc.scalar
                    dma_eng.dma_start(
                        out=outT[cb * P:(cb + 1) * P, ic * 512:(ic + 1) * 512],
                        in_=osb,
                    )
    nc.compile()
    return nc


_NC = None


def _get_nc():
    global _NC
    if _NC is None:
        _NC = _build_nc()
    return _NC


# ------------------------------------------------------------------- host ---

def _skew(t):
    z = np.zeros_like(t[:, 0])
    return np.stack([
        np.stack([z, -t[:, 2], t[:, 1]], -1),
        np.stack([t[:, 2], z, -t[:, 0]], -1),
        np.stack([-t[:, 1], t[:, 0], z], -1),
    ], 1)


def _fundamental(K1, K2, R, t):
    E = _skew(t) @ R
    U, S, Vt = np.linalg.svd(E)
    S = S.copy()
    S[:, 2] = 0.0
    E = U @ (S[:, :, None] * Vt)
    return np.linalg.inv(np.swapaxes(K2, 1, 2)) @ E @ np.linalg.inv(K1)


def _split3(v):
    """Triple bf16 split: v ~= hi + mid + lo (24 mantissa bits)."""
    v = v.astype(np.float32)
    hi = v.astype(BFNP)
    r1 = v - hi.astype(np.float32)
    mid = r1.astype(BFNP)
    r2 = r1 - mid.astype(np.float32)
    lo = r2.astype(BFNP)
    return hi, mid, lo


def _host_prep(f_src, K1, K2, R, t):
    ix, iy = np.meshgrid(np.arange(H, dtype=np.float32),
                         np.arange(W, dtype=np.float32), indexing="ij")
    x = ix.ravel()
    y = iy.ravel()
    comb = np.stack([x, y, np.ones(N, np.float32)], 0)  # (3,N)

    F = _fundamental(K1, K2, R, t)                    # (B,3,3)
    lines = (F @ comb).astype(np.float32)             # (B,3,N)
    lines = lines / lines[:, 2:3, :]
    y0 = -lines[:, 2, :] / lines[:, 1, :]
    y1 = -(lines[:, 2, :] + lines[:, 0, :] * np.float32(W)) / lines[:, 1, :]
    dy = y0 - y1
    L = np.sqrt(np.float32(W * W) + dy * dy)
    A5 = np.float32(5.0) * (dy / L)
    B5 = np.float32(5.0) * (np.float32(W) / L)
    C5 = np.float32(-5.0) * (np.float32(W) * y0 / L)

    Ah, Am, Al = _split3(A5)
    Bh, Bm, Bl = _split3(B5)
    Ch, Cm, Cl = _split3(C5)
    abc9 = np.stack([Ah, Bh, Ch, Am, Bm, Cm, Al, Bl, Cl], axis=1)  # (B,9,N)
    xy9 = np.tile(comb, (3, 1)).astype(BFNP)                        # (9,N)

    # Lipschitz bound on the row max: |grad d5| = 5 exactly, so
    # Mhat_i = max_j d5(coarse pt) + 5*dist is within [m_i, m_i + 56.6].
    gx = np.array([4., 12., 20., 28.], np.float32)
    cgx, cgy = np.meshgrid(gx, gx, indexing="ij")
    cgx = cgx.ravel()[:, None]
    cgy = cgy.ravel()[:, None]                                       # (16,1)
    dc = np.abs(A5[:, None, :] * cgx[None] + B5[:, None, :] * cgy[None]
                + C5[:, None, :])                                    # (B,16,N)
    mc = dc.max(-1)                                                  # (B,16)
    d2 = (x[None, :] - cgx) ** 2 + (y[None, :] - cgy) ** 2           # (16,N)
    near = np.argmin(d2, axis=0)                                     # (N,)
    dist = np.sqrt(d2[near, np.arange(N)])
    Mhat = mc[:, near] + np.float32(5.0) * dist[None, :]             # (B,N)

    mneg = -Mhat.reshape(B, NT, P).transpose(0, 2, 1)                # (B,128,8)
    mofs = (Mhat - np.float32(127.0 * LN2)).reshape(B, NT, P).transpose(0, 2, 1)

    fs = f_src.reshape(B, C, N).transpose(0, 2, 1)                   # (B,N,C)
    fs8 = np.clip(fs, -240, 240).astype(F8NP)
    Fcol = fs.astype(np.float64).sum(axis=1) * (SCALE / N)           # (B,C)
    f14 = Fcol.astype(np.float32).reshape(B, CB, P).transpose(0, 2, 1)  # (B,128,9)
    smallc = np.concatenate([mneg, mofs, f14], axis=2).astype(np.float32)
    return abc9, xy9, smallc, fs8


_ONEH = None
_IDENT = None


def _consts():
    global _ONEH, _IDENT
    if _ONEH is None:
        # (8, 512): row s*4+k has 1s in cols k*128..(k+1)*128, s in {hi,lo}
        oneh = np.zeros((NT, 4 * P), BFNP)
        for s in range(2):
            for k in range(4):
                oneh[s * 4 + k, k * P:(k + 1) * P] = 1.0
        _ONEH = oneh
        _IDENT = np.eye(P, dtype=BFNP)
    return _ONEH, _IDENT


def host_prep_all(f_src, K1, K2, R, t):
    abc9, xy9, smallc, fs8 = _host_prep(f_src, K1, K2, R, t)
    oneh, ident = _consts()
    in_maps = [
        {"abc9": np.ascontiguousarray(abc9[b]), "xy9": xy9,
         "smallc": np.ascontiguousarray(smallc[b]),
         "oneh": oneh, "ident": ident,
         "fs8": np.ascontiguousarray(fs8[b])}
        for b in range(B)
    ]
    return in_maps


def finish(outT_list):
    outs = np.stack([o.astype(np.float32).T for o in outT_list], 0)  # (B,N,C)
    outs *= np.float32(1.0 / SCALE)
    return outs.reshape(B, C, H, W)


def kernel(f_tar=None, f_src=None, K1=None, K2=None, R=None, t=None):
    global LAST_RESULTS
    f_src = np.asarray(f_src, np.float32)
    K1 = np.asarray(K1, np.float32)
    K2 = np.asarray(K2, np.float32)
    R = np.asarray(R, np.float32)
    t = np.asarray(t, np.float32)

    in_maps = host_prep_all(f_src, K1, K2, R, t)
    res = run_bass_kernel_spmd(_get_nc(), in_maps, list(range(B)), trace=TRACE)
    LAST_RESULTS = res
    return finish([res.results[b]["outT"] for b in range(B)])


# revision 14
# speedup vs baseline: 1.7196x; 1.7196x over previous
"""Epipolar attention kernel for Trainium2 (8 NeuronCores, batch-parallel).

Host does O(B*3^3) geometry + O(N)/O(N*C) input prep (coefficient splits,
fp8 quantization of f_src, colsum, Lipschitz row-max bound); the device does
all O(N^2) / O(N^2*C) work:

  stage1 (i-rows on partitions):
      d[i,j]  = A_j x_i + B_j y_i + C_j        (PE, exact triple-bf16 split)
      xa      = |d|                            (DVE bitcast sign-mask)
      rhat_i  = sum_j exp(xa - Mhat_i)         (ACT, per-partition bias + accum)
  beta_i = Mhat_i + ln rhat_i                  (DVE exponent-field ln, batched
      4 stripes at a time so it overlaps the stage-1 ACT tail)
      broadcast beta across partitions         (PE transpose + one-hot matmuls)
  stage2 (j-rows on partitions):
      xT      = |dT| - beta_i                  (DVE subtract; abs interleaved)
      u       = exp(xT)                        (ACT)          == e/r, exact softmax
      E2T     = exp(-u), S_j = sum_i E2T       (ACT + accum)
      W8      = E2T*(2^14/S_j) - 16            (GpSimd -> fp8e4) [attn = 1/N + W/2^14]
  stage3 (c-blocks on partitions, outT = fs^T @ W^T):
      psum[c,i] = sum_j fs8[j,c] * W8[j,i]     (PE, fp8 DoubleRow, K=256/mm)
      outT = f16(psum + 2^14*F_c/N)            (ACT Identity / DVE alternating)
Host: out = outT.T * 2^-14. The double softmax identity:
  softmax_i(1 - softmax_j(5(d-thre))) == E2/colsum(E2).
"""

import numpy as np
import ml_dtypes

import concourse.bass as bass
import concourse.bacc as bacc
import concourse.tile as tile
from concourse import mybir
from concourse.bass_utils import run_bass_kernel_spmd

# --- custom DVE op: out = |in0| - in1 (fused abs+subtract, PSUM-in) -------
from concourse import dve_ops as _dvo
from concourse.dve_spec import Spec as _Spec, Src0 as _Src0, Src1 as _Src1, \
    Zero as _Zero, maxx as _maxx, lower as _dve_lower
from concourse.dve_uop import DveOpSpec as _DveOpSpec

_ABS_SUB_NAME = "ABS_SUB_EPI"


def _register_abs_sub():
    if _ABS_SUB_NAME in _dvo._SUB_OPCODE_FOR_NAME:
        return next(op for op in _dvo.OPS if op.name == _ABS_SUB_NAME)
    spec = _Spec(
        body=_maxx(_Src0, _Zero - _Src0) - _Src1,
        reference=lambda in0, in1, s0, s1, imm2:
            np.abs(in0.astype(np.float32)) - in1,
    )
    _dvo._SUB_OPCODE_FOR_NAME[_ABS_SUB_NAME] = (
        max(_dvo._SUB_OPCODE_FOR_NAME.values()) + 1)
    shas = {}
    for ver in ("v3", "v4"):
        s = _DveOpSpec(name=_ABS_SUB_NAME,
                       opcode=_dvo.get_dve_sub_opcode(_ABS_SUB_NAME),
                       uops=_dve_lower(spec, ver=ver), rd1_en=True)
        shas[ver] = s.sha(ver)
    op = _dvo.DveOp(_ABS_SUB_NAME, spec, subdim=False, uops_sha=shas)
    _dvo.OPS.append(op)
    _dvo.CUSTOM_DVE_SPECS[_ABS_SUB_NAME] = spec
    return op


_ABS_SUB = _register_abs_sub()

B, C, H, W = 8, 1152, 32, 32
N = H * W           # 1024
P = 128
NT = N // P         # 8
CB = C // P         # 9
F32 = mybir.dt.float32
F16 = mybir.dt.float16
BF16 = mybir.dt.bfloat16
FP8 = mybir.dt.float8e4
I32 = mybir.dt.int32
BFNP = ml_dtypes.bfloat16
F8NP = ml_dtypes.float8_e4m3

SCALE = 16384.0     # 2^14 scaling of the centered attention W
LN2 = 0.6931471805599453

# log2(m) ~ c3*(m-r1)*(m^2 + pm*m + q) on [1,2]  (factored minimax cubic)
_LOG2_COEF = np.polyfit(np.cos(np.pi * (np.arange(64) + .5) / 64) * .5 + 1.5,
                        np.log2(np.cos(np.pi * (np.arange(64) + .5) / 64) * .5 + 1.5), 3)
_C3, _C2, _C1, _C0 = [float(v) for v in _LOG2_COEF]
_ROOTS = np.roots(_LOG2_COEF)
_R1 = float(_ROOTS[np.argmin(np.abs(_ROOTS.imag))].real)     # real root (~1.0)
_QUAD = np.real(np.poly([r for r in _ROOTS
                         if abs(r - _R1) > 1e-9]))            # m^2 + pm*m + q
_PM, _Q = float(_QUAD[1]), float(_QUAD[2])

TRACE = False
LAST_RESULTS = None


# ----------------------------------------------------------------- device ---

def _build_nc():
    nc = bacc.Bacc()
    abc9 = nc.dram_tensor("abc9", (9, N), BF16, kind="ExternalInput")
    xy9 = nc.dram_tensor("xy9", (9, N), BF16, kind="ExternalInput")
    smallc = nc.dram_tensor("smallc", (P, NT + NT + CB), F32,
                            kind="ExternalInput")  # [-Mhat | Mhat-127*ln2 | F*2^14/N]
    oneh = nc.dram_tensor("oneh", (NT, 4 * P), BF16, kind="ExternalInput")
    identD = nc.dram_tensor("ident", (P, P), BF16, kind="ExternalInput")
    fs8 = nc.dram_tensor("fs8", (N, C), FP8, kind="ExternalInput")
    outT = nc.dram_tensor("outT", (C, N), F16, kind="ExternalOutput")

    AF = mybir.ActivationFunctionType
    AO = mybir.AluOpType

    with tile.TileContext(nc) as tc:
        with (
            tc.tile_pool(name="consts", bufs=1) as consts,
            tc.tile_pool(name="persist", bufs=1) as persist,
            tc.tile_pool(name="pxa", bufs=3) as pxa,
            tc.tile_pool(name="pxT", bufs=3) as pxT,
            tc.tile_pool(name="pscr", bufs=1) as pscr,
            tc.tile_pool(name="pu", bufs=3) as pu,
            tc.tile_pool(name="pe2", bufs=6) as pe2,
            tc.tile_pool(name="posb", bufs=4) as posb,
            tc.tile_pool(name="stats", bufs=8) as stats,
            tc.tile_pool(name="P1", bufs=2, space="PSUM") as P1,
            tc.tile_pool(name="psC", bufs=4, space="PSUM") as psC,
        ):
            # ---- consts / inputs ----
            xy_sb = consts.tile([9, N], BF16, tag="xy")
            nc.sync.dma_start(out=xy_sb, in_=xy9[:, :])
            abc_sb = consts.tile([9, N], BF16, tag="abc")
            nc.scalar.dma_start(out=abc_sb[:, :512], in_=abc9[:, :512])
            nc.scalar.dma_start(out=abc_sb[:, 512:], in_=abc9[:, 512:])
            small_sb = consts.tile([P, NT + NT + CB], F32, tag="smallc")
            nc.sync.dma_start(out=small_sb, in_=smallc[:, :])
            mneg_sb = small_sb[:, 0:NT]
            mofs_sb = small_sb[:, NT:2 * NT]
            f14_sb = small_sb[:, 2 * NT:2 * NT + CB]
            oneh_sb = consts.tile([NT, 4 * P], BF16, tag="oneh")
            nc.scalar.dma_start(out=oneh_sb, in_=oneh[:, :])
            ident = consts.tile([P, P], BF16, tag="ident")
            nc.scalar.dma_start(out=ident, in_=identD[:, :])

            fs8_sb = persist.tile([P, NT, C], FP8, tag="fs8")
            for s in range(NT):
                nc.sync.dma_start(out=fs8_sb[:, s, :], in_=fs8[s * P:(s + 1) * P, :])

            W8_sb = persist.tile([P, NT, N], FP8, tag="W8")
            Bm_sb = persist.tile([P, N], F32, tag="Bm")
            bTa_sb = persist.tile([NT, P], BF16, tag="bTa")
            bTb_sb = persist.tile([NT, P], BF16, tag="bTb")
            bc3a = persist.tile([P, NT], BF16, tag="bc3a")
            bc3b = persist.tile([P, NT], BF16, tag="bc3b")
            rcol = persist.tile([P, NT], F32, tag="rcol")
            bcola = persist.tile([P, 4], F32, tag="bcola")
            bcolb = persist.tile([P, 4], F32, tag="bcolb")

            # preload the Exp ACT table before inputs land
            dummy = stats.tile([1, 2], F32, tag="dummy")
            nc.gpsimd.memset(dummy, 0.0)
            dummy2 = stats.tile([1, 2], F32, tag="dummy2")
            nc.scalar.activation(out=dummy2, in_=dummy, func=AF.Exp)

            # ---- stage 1: i-oriented pass -> rhat ----
            for it in range(NT):
                psd = P1.tile([P, N], F32, tag="psd")
                for h in range(2):
                    nc.tensor.matmul(
                        psd[:, h * 512:(h + 1) * 512],
                        lhsT=xy_sb[:, it * P:(it + 1) * P],
                        rhs=abc_sb[:, h * 512:(h + 1) * 512],
                        start=True, stop=True,
                    )
                xa = pxa.tile([P, N], F32, tag="xa")
                nc.vector.tensor_scalar(
                    out=xa.bitcast(I32), in0=psd.bitcast(I32),
                    scalar1=0x7FFFFFFF, scalar2=None, op0=AO.bitwise_and,
                )
                e_scr = pscr.tile([P, N], BF16, tag="escr")
                nc.scalar.activation(
                    out=e_scr, in_=xa, func=AF.Exp, bias=mneg_sb[:, it:it + 1],
                    accum_out=rcol[:, it:it + 1],
                )

            # ---- beta = Mhat + ln(rhat), exponent-field log2; two batches of
            # 4 stripes so batch A overlaps the stage-1 ACT tail ----
            def emit_beta(sl, bcol):
                rI = rcol[:, sl].bitcast(I32)
                expI = stats.tile([P, 4], I32, tag="expI", name=f"expI{sl.start}")
                nc.vector.tensor_scalar(
                    out=expI, in0=rI, scalar1=23, scalar2=None,
                    op0=AO.logical_shift_right,
                )
                Ef = stats.tile([P, 4], F32, tag="Ef", name=f"Ef{sl.start}")
                nc.vector.tensor_copy(Ef, expI)     # int -> float convert
                mant = stats.tile([P, 4], F32, tag="mant", name=f"mant{sl.start}")
                nc.vector.tensor_scalar(
                    out=mant.bitcast(I32), in0=rI, scalar1=0x007FFFFF,
                    scalar2=0x3F800000, op0=AO.bitwise_and, op1=AO.bitwise_or,
                )
                t2 = stats.tile([P, 4], F32, tag="t2", name=f"t2{sl.start}")
                nc.vector.scalar_tensor_tensor(
                    out=t2, in0=mant, scalar=_PM, in1=mant, op0=AO.add, op1=AO.mult)
                nc.vector.tensor_scalar(out=t2, in0=t2, scalar1=_Q, scalar2=None,
                                        op0=AO.add)
                t3 = stats.tile([P, 4], F32, tag="t3", name=f"t3{sl.start}")
                nc.vector.scalar_tensor_tensor(
                    out=t3, in0=mant, scalar=_R1, in1=t2, op0=AO.subtract,
                    op1=AO.mult)
                t4 = stats.tile([P, 4], F32, tag="t4", name=f"t4{sl.start}")
                nc.vector.scalar_tensor_tensor(
                    out=t4, in0=t3, scalar=_C3, in1=Ef, op0=AO.mult, op1=AO.add)
                nc.vector.scalar_tensor_tensor(
                    out=bcol, in0=t4, scalar=LN2, in1=mofs_sb[:, sl],
                    op0=AO.mult, op1=AO.add)

            def emit_split(bcol, bc3):
                # bc3 = [hi(4) | lo(4)] bf16 split of bcol
                nc.vector.tensor_copy(bc3[:, 0:4], bcol)
                nc.vector.tensor_tensor(out=bc3[:, 4:8], in0=bcol,
                                        in1=bc3[:, 0:4], op=AO.subtract)

            emit_beta(slice(0, 4), bcola)
            emit_split(bcola, bc3a)
            emit_beta(slice(4, 8), bcolb)
            emit_split(bcolb, bc3b)

            # PE: transpose + one-hot broadcast per batch -> psB halves
            psTa = P1.tile([NT, P], BF16, tag="psd", name="psTa")
            nc.tensor.transpose(psTa, bc3a, ident)
            nc.vector.tensor_copy(bTa_sb, psTa)
            psBa = P1.tile([P, 512], F32, tag="psd", name="psBa")
            for loc in range(4):
                nc.tensor.matmul(
                    psBa[:, loc * P:(loc + 1) * P],
                    lhsT=oneh_sb[:, loc * P:(loc + 1) * P],
                    rhs=bTa_sb,
                    start=True, stop=True,
                )
            psTb = P1.tile([NT, P], BF16, tag="psd", name="psTb")
            nc.tensor.transpose(psTb, bc3b, ident)
            nc.vector.tensor_copy(bTb_sb, psTb)
            psBb = P1.tile([P, 512], F32, tag="psd", name="psBb")
            for loc in range(4):
                nc.tensor.matmul(
                    psBb[:, loc * P:(loc + 1) * P],
                    lhsT=oneh_sb[:, loc * P:(loc + 1) * P],
                    rhs=bTb_sb,
                    start=True, stop=True,
                )
            nc.scalar.activation(out=Bm_sb[:, 0:512], in_=psBa, func=AF.Copy)
            nc.scalar.activation(out=Bm_sb[:, 512:], in_=psBb, func=AF.Copy)

            # ---- stage 2a matmuls (emitted after broadcast so the PE queue
            # can't block it) + stage 2b j-oriented softmax chain -> W8 ----
            S1s = [None] * NT
            e2s = [None] * NT
            invs14 = [None] * NT

            def emit_dT(u):
                psd = P1.tile([P, N], F32, tag="psd", name=f"psd2_{u}")
                for h in range(2):
                    nc.tensor.matmul(
                        psd[:, h * 512:(h + 1) * 512],
                        lhsT=abc_sb[:, u * P:(u + 1) * P],
                        rhs=xy_sb[:, h * 512:(h + 1) * 512],
                        start=True, stop=True,
                    )
                return psd

            def emit_exps(u, xT, half=None):
                # exp + exp(-u) with colsum accumulate
                sl = slice(None) if half is None else (
                    slice(0, 512) if half == 0 else slice(512, N))
                uT = pu.tile([P, N], BF16, tag="uT", name=f"uT{u}")
                nc.scalar.activation(out=uT[:, sl], in_=xT[:, sl], func=AF.Exp)
                if e2s[u] is None:
                    e2s[u] = pe2.tile([P, N], F16, tag="E2T", name=f"E2T{u}")
                tag = f"S1{u}" if half is None else f"S1{u}_{half}"
                S1 = stats.tile([P, 1], F32, tag="S1", name=tag)
                nc.scalar.activation(
                    out=e2s[u][:, sl], in_=uT[:, sl], func=AF.Exp, scale=-1.0,
                    accum_out=S1,
                )
                return S1

            def emit_inv(u, S1):
                S1s[u] = S1
                inv = stats.tile([P, 1], F32, tag="invS", name=f"invS{u}")
                nc.vector.reciprocal(inv, S1)
                invs14[u] = stats.tile([P, 1], F32, tag="invS14", name=f"iS14{u}")
                nc.vector.tensor_scalar_mul(invs14[u], inv, SCALE)

            def emit_w8(u, eng):
                eng.tensor_scalar(
                    out=W8_sb[:, u, :], in0=e2s[u], scalar1=invs14[u],
                    scalar2=SCALE / N, op0=AO.mult, op1=AO.subtract,
                )

            # PE emits all dT stripes up-front (cheap; paced by P1 rotation —
            # each psd tile is consumed directly from PSUM by the fused
            # ABS_SUB, so the rotation advances at the stage-2b pitch).
            # Stage-2b engine budget: DVE does everything elementwise
            # (fused |d|-Bm at ~1.18us + W8 at ~0.75us + stats < ACT's
            # 2.27us/stripe exp pitch); gpsimd is kept out (its
            # scalar-pointer ops are pathologically slow).
            psds = [emit_dT(u) for u in range(NT)]

            def emit_abssub(u, sl=slice(None)):
                nc.vector._custom_dve(
                    _ABS_SUB, out=xts[u][:, sl], in0=psds[u][:, sl],
                    in1=Bm_sb[:, sl])

            pend = [None] * NT   # S1 awaiting emit_inv
            xts = [None] * NT
            for u in range(NT):
                xts[u] = pxT.tile([P, N], BF16, tag="xT", name=f"xT{u}")
                if u == 0:
                    # split halves so ACT can start on Bm-left early
                    emit_abssub(0, slice(0, 512))
                    s0l = emit_exps(0, xts[0], half=0)
                    emit_abssub(0, slice(512, N))
                    s0r = emit_exps(0, xts[0], half=1)
                    S1 = stats.tile([P, 1], F32, tag="S1", name="S10")
                    nc.vector.tensor_tensor(out=S1, in0=s0l, in1=s0r, op=AO.add)
                    pend[0] = S1
                else:
                    emit_abssub(u)
                    pend[u] = emit_exps(u, xts[u])
                if u >= 1:
                    emit_inv(u - 1, pend[u - 1])
                    emit_w8(u - 1, nc.vector)
            emit_inv(7, pend[7])
            emit_w8(7, nc.vector)

            # ---- stage 3: outT[c,i] = sum_j fs8[j,c]*W8[j,i] (+ F term) ----
            # Chains run in groups of 4 distinct PSUM banks, s-major inside
            # the group, so sequential matmuls hit different banks (ILP).
            chains = [(cb, ic) for cb in range(CB) for ic in range(2)]
            for g0 in range(0, len(chains), 4):
                grp = chains[g0:g0 + 4]
                pss = [psC.tile([P, 512], F32, tag="oc",
                                name=f"oc{g0 + i}") for i in range(len(grp))]
                for s in range(4):
                    for (cb, ic), ps in zip(grp, pss):
                        nc.tensor.matmul(
                            ps,
                            lhsT=fs8_sb[:, 2 * s:2 * s + 2, cb * P:(cb + 1) * P],
                            rhs=W8_sb[:, 2 * s:2 * s + 2, ic * 512:(ic + 1) * 512],
                            start=(s == 0), stop=(s == 3),
                            perf_mode=mybir.MatmulPerfMode.DoubleRow,
                        )
                for (cb, ic), ps in zip(grp, pss):
                    osb = posb.tile([P, 512], F16, tag="osb")
                    k = cb * 2 + ic
                    if k % 2 == 0:
                        nc.scalar.activation(
                            out=osb, in_=ps, func=AF.Identity,
                            bias=f14_sb[:, cb:cb + 1],
                        )
                    else:
                        nc.vector.tensor_scalar(
                            out=osb, in0=ps, scalar1=f14_sb[:, cb:cb + 1],
                            scalar2=None, op0=AO.add,
                        )
                    dma_eng = nc.sync if k % 2 == 0 else nc.scalar
                    dma_eng.dma_start(
                        out=outT[cb * P:(cb + 1) * P, ic * 512:(ic + 1) * 512],
                        in_=osb,
                    )
    nc.compile()
    return nc


_NC = None


def _get_nc():
    global _NC
    if _NC is None:
        _NC = _build_nc()
    return _NC


# ------------------------------------------------------------------- host ---

def _skew(t):
    z = np.zeros_like(t[:, 0])
    return np.stack([
        np.stack([z, -t[:, 2], t[:, 1]], -1),
        np.stack([t[:, 2], z, -t[:, 0]], -1),
        np.stack([-t[:, 1], t[:, 0], z], -1),
    ], 1)


def _fundamental(K1, K2, R, t):
    E = _skew(t) @ R
    U, S, Vt = np.linalg.svd(E)
    S = S.copy()
    S[:, 2] = 0.0
    E = U @ (S[:, :, None] * Vt)
    return np.linalg.inv(np.swapaxes(K2, 1, 2)) @ E @ np.linalg.inv(K1)


def _split3(v):
    """Triple bf16 split: v ~= hi + mid + lo (24 mantissa bits)."""
    v = v.astype(np.float32)
    hi = v.astype(BFNP)
    r1 = v - hi.astype(np.float32)
    mid = r1.astype(BFNP)
    r2 = r1 - mid.astype(np.float32)
    lo = r2.astype(BFNP)
    return hi, mid, lo


def _host_prep(f_src, K1, K2, R, t):
    ix, iy = np.meshgrid(np.arange(H, dtype=np.float32),
                         np.arange(W, dtype=np.float32), indexing="ij")
    x = ix.ravel()
    y = iy.ravel()
    comb = np.stack([x, y, np.ones(N, np.float32)], 0)  # (3,N)

    F = _fundamental(K1, K2, R, t)                    # (B,3,3)
    lines = (F @ comb).astype(np.float32)             # (B,3,N)
    lines = lines / lines[:, 2:3, :]
    y0 = -lines[:, 2, :] / lines[:, 1, :]
    y1 = -(lines[:, 2, :] + lines[:, 0, :] * np.float32(W)) / lines[:, 1, :]
    dy = y0 - y1
    L = np.sqrt(np.float32(W * W) + dy * dy)
    A5 = np.float32(5.0) * (dy / L)
    B5 = np.float32(5.0) * (np.float32(W) / L)
    C5 = np.float32(-5.0) * (np.float32(W) * y0 / L)

    Ah, Am, Al = _split3(A5)
    Bh, Bm, Bl = _split3(B5)
    Ch, Cm, Cl = _split3(C5)
    abc9 = np.stack([Ah, Bh, Ch, Am, Bm, Cm, Al, Bl, Cl], axis=1)  # (B,9,N)
    xy9 = np.tile(comb, (3, 1)).astype(BFNP)                        # (9,N)

    # Lipschitz bound on the row max: |grad d5| = 5 exactly, so
    # Mhat_i = max_j d5(coarse pt) + 5*dist is within [m_i, m_i + 56.6].
    gx = np.array([4., 12., 20., 28.], np.float32)
    cgx, cgy = np.meshgrid(gx, gx, indexing="ij")
    cgx = cgx.ravel()[:, None]
    cgy = cgy.ravel()[:, None]                                       # (16,1)
    dc = np.abs(A5[:, None, :] * cgx[None] + B5[:, None, :] * cgy[None]
                + C5[:, None, :])                                    # (B,16,N)
    mc = dc.max(-1)                                                  # (B,16)
    d2 = (x[None, :] - cgx) ** 2 + (y[None, :] - cgy) ** 2           # (16,N)
    near = np.argmin(d2, axis=0)                                     # (N,)
    dist = np.sqrt(d2[near, np.arange(N)])
    Mhat = mc[:, near] + np.float32(5.0) * dist[None, :]             # (B,N)

    mneg = -Mhat.reshape(B, NT, P).transpose(0, 2, 1)                # (B,128,8)
    mofs = (Mhat - np.float32(127.0 * LN2)).reshape(B, NT, P).transpose(0, 2, 1)

    fs = f_src.reshape(B, C, N).transpose(0, 2, 1)                   # (B,N,C)
    fs8 = np.clip(fs, -240, 240).astype(F8NP)
    Fcol = fs.astype(np.float64).sum(axis=1) * (SCALE / N)           # (B,C)
    f14 = Fcol.astype(np.float32).reshape(B, CB, P).transpose(0, 2, 1)  # (B,128,9)
    smallc = np.concatenate([mneg, mofs, f14], axis=2).astype(np.float32)
    return abc9, xy9, smallc, fs8


_ONEH = None
_IDENT = None


def _consts():
    global _ONEH, _IDENT
    if _ONEH is None:
        # (8, 512): row s*4+k has 1s in cols k*128..(k+1)*128, s in {hi,lo}
        oneh = np.zeros((NT, 4 * P), BFNP)
        for s in range(2):
            for k in range(4):
                oneh[s * 4 + k, k * P:(k + 1) * P] = 1.0
        _ONEH = oneh
        _IDENT = np.eye(P, dtype=BFNP)
    return _ONEH, _IDENT


def host_prep_all(f_src, K1, K2, R, t):
    abc9, xy9, smallc, fs8 = _host_prep(f_src, K1, K2, R, t)
    oneh, ident = _consts()
    in_maps = [
        {"abc9": np.ascontiguousarray(abc9[b]), "xy9": xy9,
         "smallc": np.ascontiguousarray(smallc[b]),
         "oneh": oneh, "ident": ident,
         "fs8": np.ascontiguousarray(fs8[b])}
        for b in range(B)
    ]
    return in_maps


def finish(outT_list):
    outs = np.stack([o.astype(np.float32).T for o in outT_list], 0)  # (B,N,C)
    outs *= np.float32(1.0 / SCALE)
    return outs.reshape(B, C, H, W)


def kernel(f_tar=None, f_src=None, K1=None, K2=None, R=None, t=None):
    global LAST_RESULTS
    f_src = np.asarray(f_src, np.float32)
    K1 = np.asarray(K1, np.float32)
    K2 = np.asarray(K2, np.float32)
    R = np.asarray(R, np.float32)
    t = np.asarray(t, np.float32)

    in_maps = host_prep_all(f_src, K1, K2, R, t)
    res = run_bass_kernel_spmd(_get_nc(), in_maps, list(range(B)), trace=TRACE)
    LAST_RESULTS = res
    return finish([res.results[b]["outT"] for b in range(B)])


# revision 16
# speedup vs baseline: 1.8090x; 1.0520x over previous
"""Epipolar attention kernel for Trainium2 (8 NeuronCores, batch-parallel).

Host does O(B*3^3) geometry + O(N)/O(N*C) input prep (coefficient splits,
fp8 quantization of f_src, colsum, Lipschitz row-max bound Mhat broadcast);
the device does all O(N^2) / O(N^2*C) work in a single j-major orientation:

  pass 1 (j-rows on partitions):
      dT[j,i]  = A_j x_i + B_j y_i + C_j       (PE, exact triple-bf16 split)
      xT'      = |dT| - Mhat_i                 (fused custom DVE op, PSUM in)
      u'       = exp(xT')                      (ACT)     == softmax numerator
      rhat_i   = sum_j u'[j,i]                 (PE ones-matmul column sum)
  rinv broadcast: rhat -> (1,N) -> PE one-row broadcast -> fast-reciprocal
  pass 2:
      u        = u' * rinv_i                   (DVE / GpSimd)  == exact softmax
      E2T      = exp(-u), S_j = sum_i E2T      (ACT + accum)
      W8       = E2T*(2^14/S_j) - 16           (DVE -> fp8e4) [attn = 1/N + W/2^14]
  stage 3 (c-blocks on partitions, outT = fs^T @ W^T):
      psum[c,i] = sum_j fs8[j,c] * W8[j,i]     (PE, fp8 DoubleRow, K=256/mm)
      outT = f16(psum + 2^14*F_c/N)            (ACT Identity / DVE, + bias)
Host: out = outT.T * 2^-14. The double softmax identity:
  softmax_i(1 - softmax_j(5(d-thre))) == E2/colsum(E2).
"""

import numpy as np
import ml_dtypes

import concourse.bass as bass
import concourse.bacc as bacc
import concourse.tile as tile
from concourse import mybir
from concourse.bass_utils import run_bass_kernel_spmd

# --- custom DVE op: out = |in0| - in1 (fused abs+subtract, PSUM-in) -------
from concourse import dve_ops as _dvo
from concourse.dve_ops import RECIPROCAL_APPROX_FAST, RECIP_APPROX_FAST_CONSTS
from concourse.dve_spec import Spec as _Spec, Src0 as _Src0, Src1 as _Src1, \
    Zero as _Zero, maxx as _maxx, lower as _dve_lower
from concourse.dve_uop import DveOpSpec as _DveOpSpec

_ABS_SUB_NAME = "ABS_SUB_EPI"


def _register_abs_sub():
    if _ABS_SUB_NAME in _dvo._SUB_OPCODE_FOR_NAME:
        return next(op for op in _dvo.OPS if op.name == _ABS_SUB_NAME)
    spec = _Spec(
        body=_maxx(_Src0, _Zero - _Src0) - _Src1,
        reference=lambda in0, in1, s0, s1, imm2:
            np.abs(in0.astype(np.float32)) - in1,
    )
    _dvo._SUB_OPCODE_FOR_NAME[_ABS_SUB_NAME] = (
        max(_dvo._SUB_OPCODE_FOR_NAME.values()) + 1)
    shas = {}
    for ver in ("v3", "v4"):
        s = _DveOpSpec(name=_ABS_SUB_NAME,
                       opcode=_dvo.get_dve_sub_opcode(_ABS_SUB_NAME),
                       uops=_dve_lower(spec, ver=ver), rd1_en=True)
        shas[ver] = s.sha(ver)
    op = _dvo.DveOp(_ABS_SUB_NAME, spec, subdim=False, uops_sha=shas)
    _dvo.OPS.append(op)
    _dvo.CUSTOM_DVE_SPECS[_ABS_SUB_NAME] = spec
    return op


_ABS_SUB = _register_abs_sub()

B, C, H, W = 8, 1152, 32, 32
N = H * W           # 1024
P = 128
NT = N // P         # 8
CB = C // P         # 9
F32 = mybir.dt.float32
F16 = mybir.dt.float16
BF16 = mybir.dt.bfloat16
FP8 = mybir.dt.float8e4
I32 = mybir.dt.int32
BFNP = ml_dtypes.bfloat16
F8NP = ml_dtypes.float8_e4m3

SCALE = 16384.0     # 2^14 scaling of the centered attention W
HB = 512            # psum-bank-sized half width

TRACE = False
LAST_RESULTS = None


# ----------------------------------------------------------------- device ---

def _build_nc():
    nc = bacc.Bacc()
    abc9 = nc.dram_tensor("abc9", (9, N), BF16, kind="ExternalInput")
    xy9 = nc.dram_tensor("xy9", (9, N), BF16, kind="ExternalInput")
    mhatB = nc.dram_tensor("mhatB", (P, N), F32, kind="ExternalInput")
    f14c = nc.dram_tensor("f14c", (P, CB), F32, kind="ExternalInput")
    fs8 = nc.dram_tensor("fs8", (N, C), FP8, kind="ExternalInput")
    outT = nc.dram_tensor("outT", (C, N), F16, kind="ExternalOutput")

    AF = mybir.ActivationFunctionType
    AO = mybir.AluOpType

    with tile.TileContext(nc) as tc:
        with (
            tc.tile_pool(name="consts", bufs=1) as consts,
            tc.tile_pool(name="persist", bufs=1) as persist,
            tc.tile_pool(name="pxT", bufs=3) as pxT,
            tc.tile_pool(name="pum", bufs=3) as pum,
            tc.tile_pool(name="pu", bufs=3) as pu,
            tc.tile_pool(name="pe2", bufs=6) as pe2,
            tc.tile_pool(name="posb", bufs=3) as posb,
            tc.tile_pool(name="stats", bufs=8) as stats,
        ):
            # ---- consts / inputs ----
            xy_sb = consts.tile([9, N], BF16, tag="xy")
            nc.sync.dma_start(out=xy_sb, in_=xy9[:, :])
            MhB_sb = consts.tile([P, N], F32, tag="MhB")
            nc.sync.dma_start(out=MhB_sb[:, :HB], in_=mhatB[:, :HB])
            nc.sync.dma_start(out=MhB_sb[:, HB:], in_=mhatB[:, HB:])
            abc_sb = consts.tile([9, N], BF16, tag="abc")
            nc.scalar.dma_start(out=abc_sb[:, :HB], in_=abc9[:, :HB])
            nc.scalar.dma_start(out=abc_sb[:, HB:], in_=abc9[:, HB:])
            f14_sb = consts.tile([P, CB], F32, tag="f14")
            nc.scalar.dma_start(out=f14_sb, in_=f14c[:, :])

            fs8_sb = persist.tile([P, NT, C], FP8, tag="fs8")
            for s in range(NT):
                nc.sync.dma_start(out=fs8_sb[:, s, :], in_=fs8[s * P:(s + 1) * P, :])

            W8_sb = persist.tile([P, NT, N], FP8, tag="W8")
            up_sb = persist.tile([P, NT, N], BF16, tag="up")    # u' storage
            RinvB = persist.tile([P, N], F32, tag="RinvB")
            rt_sb = persist.tile([1, N], F32, tag="rt")
            onesK = persist.tile([P, 1], BF16, tag="onesK")     # colsum lhsT
            nc.gpsimd.memset(onesK, 1.0)
            ones1 = persist.tile([1, P], F32, tag="ones1")      # bcast lhsT
            nc.gpsimd.memset(ones1, 1.0)

            # preload the Exp ACT table before inputs land
            dummy = stats.tile([1, 2], F32, tag="dummy")
            nc.gpsimd.memset(dummy, 0.0)
            dummy2 = stats.tile([1, 2], F32, tag="dummy2")
            nc.scalar.activation(out=dummy2, in_=dummy, func=AF.Exp)

            xts = [None] * NT

            # ---- pass 1: dT -> |dT|-Mhat -> u' = exp(.) -> rhat colsum ----
            with (
                tc.tile_pool(name="P1", bufs=2, space="PSUM") as P1,
                tc.tile_pool(name="prt", bufs=1, space="PSUM") as prt,
            ):
                rT = [prt.tile([1, HB], F32, tag=f"rt{h}", name=f"rT{h}")
                      for h in range(2)]

                def emit_dT(u, h):
                    psd = P1.tile([P, HB], F32, tag="psd", name=f"ps_{u}_{h}")
                    nc.tensor.matmul(
                        psd,
                        lhsT=abc_sb[:, u * P:(u + 1) * P],
                        rhs=xy_sb[:, h * HB:(h + 1) * HB],
                        start=True, stop=True,
                    )
                    return psd

                def emit_colsum(u, h):
                    nc.tensor.matmul(
                        rT[h],
                        lhsT=onesK,
                        rhs=up_sb[:, u, h * HB:(h + 1) * HB],
                        start=(u == 0), stop=(u == NT - 1),
                    )

                for u in range(NT):
                    xts[u] = pxT.tile([P, N], BF16, tag="xT", name=f"xT{u}")
                    for h in range(2):
                        psd = emit_dT(u, h)
                        sl = slice(h * HB, (h + 1) * HB)
                        nc.vector._custom_dve(
                            _ABS_SUB, out=xts[u][:, sl], in0=psd,
                            in1=MhB_sb[:, sl])
                    nc.scalar.activation(out=up_sb[:, u, :], in_=xts[u],
                                         func=AF.Exp)
                    # colsums trail by 2 stripes so the PE queue never waits
                    # on ACT in front of a ready d-matmul
                    if u >= 2:
                        emit_colsum(u - 2, 0)
                        emit_colsum(u - 2, 1)
                for u in (NT - 2, NT - 1):
                    emit_colsum(u, 0)
                    emit_colsum(u, 1)

                # rhat -> (1,N) SBUF -> broadcast (PE) -> fast reciprocal
                for h in range(2):
                    nc.scalar.activation(
                        out=rt_sb[:, h * HB:(h + 1) * HB], in_=rT[h],
                        func=AF.Copy)
                for h in range(2):
                    RhB = prt.tile([P, HB], F32, tag=f"rt{h}", name=f"rhb{h}")
                    nc.tensor.matmul(
                        RhB,
                        lhsT=ones1,
                        rhs=rt_sb[:, h * HB:(h + 1) * HB],
                        start=True, stop=True,
                    )
                    nc.vector._custom_dve(
                        RECIPROCAL_APPROX_FAST,
                        out=RinvB[:, h * HB:(h + 1) * HB], in0=RhB,
                        **RECIP_APPROX_FAST_CONSTS)

            # ---- pass 2 + stage 3 (PSUM banks reclaimed for accumulators) --
            with tc.tile_pool(name="psC", bufs=8, space="PSUM") as psC:
                e2s = [None] * NT
                invs14 = [None] * NT
                pend = [None] * NT

                def emit_mult(u, eng):
                    um = pum.tile([P, N], BF16, tag="um", name=f"um{u}")
                    eng.tensor_tensor(out=um, in0=up_sb[:, u, :], in1=RinvB,
                                      op=AO.mult)
                    return um

                def emit_e2(u, um):
                    e2s[u] = pe2.tile([P, N], F16, tag="E2T", name=f"E2T{u}")
                    S1 = stats.tile([P, 1], F32, tag="S1", name=f"S1{u}")
                    nc.scalar.activation(
                        out=e2s[u], in_=um, func=AF.Exp, scale=-1.0,
                        accum_out=S1,
                    )
                    return S1

                def emit_inv(u):
                    inv = stats.tile([P, 1], F32, tag="invS", name=f"invS{u}")
                    nc.vector.reciprocal(inv, pend[u])
                    invs14[u] = stats.tile([P, 1], F32, tag="invS14",
                                           name=f"iS14{u}")
                    nc.vector.tensor_scalar_mul(invs14[u], inv, SCALE)

                def emit_w8(u):
                    nc.vector.tensor_scalar(
                        out=W8_sb[:, u, :], in0=e2s[u], scalar1=invs14[u],
                        scalar2=SCALE / N, op0=AO.mult, op1=AO.subtract,
                    )

                # gpsimd takes half the u'*rinv products (tensor_tensor is
                # safe there); DVE takes the rest plus stats and W8
                for u in range(NT):
                    um = emit_mult(u, nc.gpsimd if u % 2 == 0 else nc.vector)
                    pend[u] = emit_e2(u, um)
                    if u >= 1:
                        emit_inv(u - 1)
                        emit_w8(u - 1)
                emit_inv(NT - 1)
                emit_w8(NT - 1)

                # ---- stage 3: outT[c,i] = sum_j fs8[j,c]*W8[j,i] + F ----
                osbs = {}
                sent = set()
                chains = [(cb, ic) for cb in range(CB) for ic in range(2)]
                for g0 in range(0, len(chains), 8):
                    grp = chains[g0:g0 + 8]
                    pss = [psC.tile([P, HB], F32, tag="oc",
                                    name=f"oc{g0 + i}")
                           for i in range(len(grp))]
                    for s in range(4):
                        for (cb, ic), ps in zip(grp, pss):
                            nc.tensor.matmul(
                                ps,
                                lhsT=fs8_sb[:, 2 * s:2 * s + 2,
                                            cb * P:(cb + 1) * P],
                                rhs=W8_sb[:, 2 * s:2 * s + 2,
                                          ic * HB:(ic + 1) * HB],
                                start=(s == 0), stop=(s == 3),
                                perf_mode=mybir.MatmulPerfMode.DoubleRow,
                            )
                    for (cb, ic), ps in zip(grp, pss):
                        if cb not in osbs:
                            osbs[cb] = posb.tile([P, N], F16, tag="osb",
                                                 name=f"osb{cb}")
                        osb = osbs[cb]
                        if ic == 0:
                            nc.scalar.activation(
                                out=osb[:, :HB], in_=ps, func=AF.Identity,
                                bias=f14_sb[:, cb:cb + 1],
                            )
                        else:
                            nc.vector.tensor_scalar(
                                out=osb[:, HB:], in0=ps,
                                scalar1=f14_sb[:, cb:cb + 1],
                                scalar2=None, op0=AO.add,
                            )
                    evacd = chains[:g0 + len(grp)]
                    for cb in range(CB):
                        if cb in sent:
                            continue
                        if (cb, 0) in evacd and (cb, 1) in evacd:
                            dma_eng = nc.sync if cb % 2 == 0 else nc.scalar
                            dma_eng.dma_start(
                                out=outT[cb * P:(cb + 1) * P, :],
                                in_=osbs[cb],
                            )
                            sent.add(cb)
    nc.compile()
    return nc


_NC = None


def _get_nc():
    global _NC
    if _NC is None:
        _NC = _build_nc()
    return _NC


# ------------------------------------------------------------------- host ---

def _skew(t):
    z = np.zeros_like(t[:, 0])
    return np.stack([
        np.stack([z, -t[:, 2], t[:, 1]], -1),
        np.stack([t[:, 2], z, -t[:, 0]], -1),
        np.stack([-t[:, 1], t[:, 0], z], -1),
    ], 1)


def _fundamental(K1, K2, R, t):
    E = _skew(t) @ R
    U, S, Vt = np.linalg.svd(E)
    S = S.copy()
    S[:, 2] = 0.0
    E = U @ (S[:, :, None] * Vt)
    return np.linalg.inv(np.swapaxes(K2, 1, 2)) @ E @ np.linalg.inv(K1)


def _split3(v):
    """Triple bf16 split: v ~= hi + mid + lo (24 mantissa bits)."""
    v = v.astype(np.float32)
    hi = v.astype(BFNP)
    r1 = v - hi.astype(np.float32)
    mid = r1.astype(BFNP)
    r2 = r1 - mid.astype(np.float32)
    lo = r2.astype(BFNP)
    return hi, mid, lo


def _host_prep(f_src, K1, K2, R, t):
    ix, iy = np.meshgrid(np.arange(H, dtype=np.float32),
                         np.arange(W, dtype=np.float32), indexing="ij")
    x = ix.ravel()
    y = iy.ravel()
    comb = np.stack([x, y, np.ones(N, np.float32)], 0)  # (3,N)

    F = _fundamental(K1, K2, R, t)                    # (B,3,3)
    lines = (F @ comb).astype(np.float32)             # (B,3,N)
    lines = lines / lines[:, 2:3, :]
    y0 = -lines[:, 2, :] / lines[:, 1, :]
    y1 = -(lines[:, 2, :] + lines[:, 0, :] * np.float32(W)) / lines[:, 1, :]
    dy = y0 - y1
    L = np.sqrt(np.float32(W * W) + dy * dy)
    A5 = np.float32(5.0) * (dy / L)
    B5 = np.float32(5.0) * (np.float32(W) / L)
    C5 = np.float32(-5.0) * (np.float32(W) * y0 / L)

    Ah, Am, Al = _split3(A5)
    Bh, Bm, Bl = _split3(B5)
    Ch, Cm, Cl = _split3(C5)
    abc9 = np.stack([Ah, Bh, Ch, Am, Bm, Cm, Al, Bl, Cl], axis=1)  # (B,9,N)
    xy9 = np.tile(comb, (3, 1)).astype(BFNP)                        # (9,N)

    # Lipschitz bound on the row max: |grad d5| = 5 exactly, so
    # Mhat_i = max_j d5(coarse pt) + 5*dist is within [m_i, m_i + 56.6].
    gx = np.array([4., 12., 20., 28.], np.float32)
    cgx, cgy = np.meshgrid(gx, gx, indexing="ij")
    cgx = cgx.ravel()[:, None]
    cgy = cgy.ravel()[:, None]                                       # (16,1)
    dc = np.abs(A5[:, None, :] * cgx[None] + B5[:, None, :] * cgy[None]
                + C5[:, None, :])                                    # (B,16,N)
    mc = dc.max(-1)                                                  # (B,16)
    d2 = (x[None, :] - cgx) ** 2 + (y[None, :] - cgy) ** 2           # (16,N)
    near = np.argmin(d2, axis=0)                                     # (N,)
    dist = np.sqrt(d2[near, np.arange(N)])
    Mhat = mc[:, near] + np.float32(5.0) * dist[None, :]             # (B,N)

    fs = f_src.reshape(B, C, N).transpose(0, 2, 1)                   # (B,N,C)
    fs8 = np.clip(fs, -240, 240).astype(F8NP)
    Fcol = fs.astype(np.float64).sum(axis=1) * (SCALE / N)           # (B,C)
    f14 = Fcol.astype(np.float32).reshape(B, CB, P).transpose(0, 2, 1)  # (B,128,9)
    return abc9, xy9, Mhat, f14.astype(np.float32), fs8


def host_prep_all(f_src, K1, K2, R, t):
    abc9, xy9, Mhat, f14, fs8 = _host_prep(f_src, K1, K2, R, t)
    in_maps = [
        {"abc9": np.ascontiguousarray(abc9[b]), "xy9": xy9,
         "mhatB": np.ascontiguousarray(
             np.broadcast_to(Mhat[b], (P, N)).astype(np.float32)),
         "f14c": np.ascontiguousarray(f14[b]),
         "fs8": np.ascontiguousarray(fs8[b])}
        for b in range(B)
    ]
    return in_maps


def finish(outT_list):
    outs = np.stack([o.astype(np.float32).T for o in outT_list], 0)  # (B,N,C)
    outs *= np.float32(1.0 / SCALE)
    return outs.reshape(B, C, H, W)


def kernel(f_tar=None, f_src=None, K1=None, K2=None, R=None, t=None):
    global LAST_RESULTS
    f_src = np.asarray(f_src, np.float32)
    K1 = np.asarray(K1, np.float32)
    K2 = np.asarray(K2, np.float32)
    R = np.asarray(R, np.float32)
    t = np.asarray(t, np.float32)

    in_maps = host_prep_all(f_src, K1, K2, R, t)
    res = run_bass_kernel_spmd(_get_nc(), in_maps, list(range(B)), trace=TRACE)
    LAST_RESULTS = res
    return finish([res.results[b]["outT"] for b in range(B)])


# revision 18
# speedup vs baseline: 2.0066x; 1.1092x over previous
"""Epipolar attention kernel for Trainium2 (8 NeuronCores, batch-parallel).

Host does O(B*3^3) geometry + O(N)/O(N*C) input prep (coefficient splits,
fp8 quantization of f_src, colsum, Lipschitz row-max bound Mhat broadcast);
the device does all O(N^2) / O(N^2*C) work in a single j-major orientation:

  pass 1 (j-rows on partitions):
      dT[j,i]  = A_j x_i + B_j y_i + C_j       (PE, exact triple-bf16 split)
      xT'      = |dT| - Mhat_i                 (fused custom DVE op, PSUM in)
      u'       = exp(xT')                      (ACT)     == softmax numerator
      rhat_i   = sum_j u'[j,i]                 (PE ones-matmul column sum)
  rinv broadcast: rhat -> (1,N) -> PE one-row broadcast -> fast-reciprocal
  pass 2:
      u        = u' * rinv_i                   (DVE / GpSimd)  == exact softmax
      E2T      = exp(-u), S_j = sum_i E2T      (ACT + accum)
      W8       = E2T*(2^14/S_j) - 16           (DVE -> fp8e4) [attn = 1/N + W/2^14]
  stage 3 (c-blocks on partitions, outT = fs^T @ W^T):
      psum[c,i] = sum_j fs8[j,c] * W8[j,i]     (PE, fp8 DoubleRow, K=256/mm)
      outT = f16(psum + 2^14*F_c/N)            (ACT Identity / DVE, + bias)
Host: out = outT.T * 2^-14. The double softmax identity:
  softmax_i(1 - softmax_j(5(d-thre))) == E2/colsum(E2).
"""

import numpy as np
import ml_dtypes

import concourse.bass as bass
import concourse.bacc as bacc
import concourse.tile as tile
from concourse import mybir
from concourse.bass_utils import run_bass_kernel_spmd

# --- custom DVE op: out = |in0| - in1 (fused abs+subtract, PSUM-in) -------
from concourse import dve_ops as _dvo
from concourse.dve_ops import RECIPROCAL_APPROX_FAST, RECIP_APPROX_FAST_CONSTS
from concourse.dve_spec import Spec as _Spec, Src0 as _Src0, Src1 as _Src1, \
    Zero as _Zero, maxx as _maxx, lower as _dve_lower
from concourse.dve_uop import DveOpSpec as _DveOpSpec

_ABS_SUB_NAME = "ABS_SUB_EPI"


def _register_abs_sub():
    if _ABS_SUB_NAME in _dvo._SUB_OPCODE_FOR_NAME:
        return next(op for op in _dvo.OPS if op.name == _ABS_SUB_NAME)
    spec = _Spec(
        body=_maxx(_Src0, _Zero - _Src0) - _Src1,
        reference=lambda in0, in1, s0, s1, imm2:
            np.abs(in0.astype(np.float32)) - in1,
    )
    _dvo._SUB_OPCODE_FOR_NAME[_ABS_SUB_NAME] = (
        max(_dvo._SUB_OPCODE_FOR_NAME.values()) + 1)
    shas = {}
    for ver in ("v3", "v4"):
        s = _DveOpSpec(name=_ABS_SUB_NAME,
                       opcode=_dvo.get_dve_sub_opcode(_ABS_SUB_NAME),
                       uops=_dve_lower(spec, ver=ver), rd1_en=True)
        shas[ver] = s.sha(ver)
    op = _dvo.DveOp(_ABS_SUB_NAME, spec, subdim=False, uops_sha=shas)
    _dvo.OPS.append(op)
    _dvo.CUSTOM_DVE_SPECS[_ABS_SUB_NAME] = spec
    return op


_ABS_SUB = _register_abs_sub()

B, C, H, W = 8, 1152, 32, 32
N = H * W           # 1024
P = 128
NT = N // P         # 8
CB = C // P         # 9
F32 = mybir.dt.float32
F16 = mybir.dt.float16
BF16 = mybir.dt.bfloat16
FP8 = mybir.dt.float8e4
I32 = mybir.dt.int32
BFNP = ml_dtypes.bfloat16
F8NP = ml_dtypes.float8_e4m3

SCALE = 16384.0     # 2^14 scaling of the centered attention W
HB = 512            # psum-bank-sized half width

TRACE = False
LAST_RESULTS = None


# ----------------------------------------------------------------- device ---

def _build_nc():
    nc = bacc.Bacc()
    abc9 = nc.dram_tensor("abc9", (9, N), BF16, kind="ExternalInput")
    xy9 = nc.dram_tensor("xy9", (9, N), BF16, kind="ExternalInput")
    mhatB = nc.dram_tensor("mhatB", (P, N), F32, kind="ExternalInput")
    f14c = nc.dram_tensor("f14c", (P, CB), F32, kind="ExternalInput")
    fs8 = nc.dram_tensor("fs8", (N, C), FP8, kind="ExternalInput")
    outT = nc.dram_tensor("outT", (C, N), F16, kind="ExternalOutput")

    AF = mybir.ActivationFunctionType
    AO = mybir.AluOpType

    with tile.TileContext(nc) as tc:
        with (
            tc.tile_pool(name="consts", bufs=1) as consts,
            tc.tile_pool(name="persist", bufs=1) as persist,
            tc.tile_pool(name="pxT", bufs=3) as pxT,
            tc.tile_pool(name="pum", bufs=3) as pum,
            tc.tile_pool(name="pu", bufs=3) as pu,
            tc.tile_pool(name="pe2", bufs=6) as pe2,
            tc.tile_pool(name="posb", bufs=3) as posb,
            tc.tile_pool(name="stats", bufs=8) as stats,
        ):
            # ---- consts / inputs ----
            xy_sb = consts.tile([9, N], BF16, tag="xy")
            nc.sync.dma_start(out=xy_sb, in_=xy9[:, :])
            MhB_sb = consts.tile([P, N], F32, tag="MhB")
            nc.sync.dma_start(out=MhB_sb[:, :HB], in_=mhatB[:, :HB])
            nc.sync.dma_start(out=MhB_sb[:, HB:], in_=mhatB[:, HB:])
            abc_sb = consts.tile([9, N], BF16, tag="abc")
            nc.scalar.dma_start(out=abc_sb[:, :HB], in_=abc9[:, :HB])
            nc.scalar.dma_start(out=abc_sb[:, HB:], in_=abc9[:, HB:])
            f14_sb = consts.tile([P, CB], F32, tag="f14")
            nc.scalar.dma_start(out=f14_sb, in_=f14c[:, :])

            fs8_sb = persist.tile([P, NT, C], FP8, tag="fs8")
            for s in range(NT):
                nc.sync.dma_start(out=fs8_sb[:, s, :], in_=fs8[s * P:(s + 1) * P, :])

            W8_sb = persist.tile([P, NT, N], FP8, tag="W8")
            up_sb = persist.tile([P, NT, N], BF16, tag="up")    # u' storage
            RinvB = persist.tile([P, N], F32, tag="RinvB")
            rt_sb = persist.tile([1, N], BF16, tag="rt")
            onesK = persist.tile([P, 1], BF16, tag="onesK")     # colsum lhsT
            nc.gpsimd.memset(onesK, 1.0)
            ones1 = persist.tile([1, P], BF16, tag="ones1")     # bcast lhsT
            nc.gpsimd.memset(ones1, 1.0)

            # preload the Exp ACT table before inputs land
            dummy = stats.tile([1, 2], F32, tag="dummy")
            nc.gpsimd.memset(dummy, 0.0)
            dummy2 = stats.tile([1, 2], F32, tag="dummy2")
            nc.scalar.activation(out=dummy2, in_=dummy, func=AF.Exp)

            xts = [None] * NT

            # ---- pass 1: dT -> |dT|-Mhat -> u' = exp(.) -> rhat colsum ----
            with (
                tc.tile_pool(name="P1", bufs=2, space="PSUM") as P1,
                tc.tile_pool(name="prt", bufs=1, space="PSUM") as prt,
            ):
                rT = [prt.tile([1, HB], F32, tag=f"rt{h}", name=f"rT{h}")
                      for h in range(2)]

                def emit_dT(u, h):
                    psd = P1.tile([P, HB], F32, tag="psd", name=f"ps_{u}_{h}")
                    nc.tensor.matmul(
                        psd,
                        lhsT=abc_sb[:, u * P:(u + 1) * P],
                        rhs=xy_sb[:, h * HB:(h + 1) * HB],
                        start=True, stop=True,
                    )
                    return psd

                def emit_colsum(u, h):
                    nc.tensor.matmul(
                        rT[h],
                        lhsT=onesK,
                        rhs=up_sb[:, u, h * HB:(h + 1) * HB],
                        start=(u == 0), stop=(u == NT - 1),
                    )

                for u in range(NT):
                    xts[u] = pxT.tile([P, N], BF16, tag="xT", name=f"xT{u}")
                    for h in range(2):
                        psd = emit_dT(u, h)
                        sl = slice(h * HB, (h + 1) * HB)
                        nc.vector._custom_dve(
                            _ABS_SUB, out=xts[u][:, sl], in0=psd,
                            in1=MhB_sb[:, sl])
                    nc.scalar.activation(out=up_sb[:, u, :], in_=xts[u],
                                         func=AF.Exp)
                    # colsums trail by 2 stripes so the PE queue never waits
                    # on ACT in front of a ready d-matmul
                    if u >= 2:
                        emit_colsum(u - 2, 0)
                        emit_colsum(u - 2, 1)
                for u in (NT - 2, NT - 1):
                    emit_colsum(u, 0)
                    emit_colsum(u, 1)

                # rhat -> (1,N) SBUF -> broadcast (PE) -> fast reciprocal
                for h in range(2):
                    nc.scalar.activation(
                        out=rt_sb[:, h * HB:(h + 1) * HB], in_=rT[h],
                        func=AF.Copy)
                for h in range(2):
                    RhB = prt.tile([P, HB], F32, tag=f"rt{h}", name=f"rhb{h}")
                    nc.tensor.matmul(
                        RhB,
                        lhsT=ones1,
                        rhs=rt_sb[:, h * HB:(h + 1) * HB],
                        start=True, stop=True,
                    )
                    nc.vector._custom_dve(
                        RECIPROCAL_APPROX_FAST,
                        out=RinvB[:, h * HB:(h + 1) * HB], in0=RhB,
                        **RECIP_APPROX_FAST_CONSTS)

            # ---- pass 2 + stage 3 (PSUM banks reclaimed for accumulators) --
            with tc.tile_pool(name="psC", bufs=8, space="PSUM") as psC:
                e2s = [None] * NT
                invs14 = [None] * NT
                pend = [None] * NT

                def emit_mult(u, eng):
                    um = pum.tile([P, N], BF16, tag="um", name=f"um{u}")
                    eng.tensor_tensor(out=um, in0=up_sb[:, u, :], in1=RinvB,
                                      op=AO.mult)
                    return um

                def emit_e2(u, um):
                    e2s[u] = pe2.tile([P, N], F16, tag="E2T", name=f"E2T{u}")
                    S1 = stats.tile([P, 1], F32, tag="S1", name=f"S1{u}")
                    nc.scalar.activation(
                        out=e2s[u], in_=um, func=AF.Exp, scale=-1.0,
                        accum_out=S1,
                    )
                    return S1

                def emit_inv(u):
                    inv = stats.tile([P, 1], F32, tag="invS", name=f"invS{u}")
                    nc.vector.reciprocal(inv, pend[u])
                    invs14[u] = stats.tile([P, 1], F32, tag="invS14",
                                           name=f"iS14{u}")
                    nc.vector.tensor_scalar_mul(invs14[u], inv, SCALE)

                def emit_w8(u):
                    nc.vector.tensor_scalar(
                        out=W8_sb[:, u, :], in0=e2s[u], scalar1=invs14[u],
                        scalar2=SCALE / N, op0=AO.mult, op1=AO.subtract,
                    )

                # everything elementwise on DVE — gpsimd's big tensor ops
                # stall concurrent DVE instructions (observed on hw)
                for u in range(NT):
                    um = emit_mult(u, nc.vector)
                    pend[u] = emit_e2(u, um)
                    if u >= 1:
                        emit_inv(u - 1)
                        emit_w8(u - 1)
                emit_inv(NT - 1)
                emit_w8(NT - 1)

                # ---- stage 3: outT[c,i] = sum_j fs8[j,c]*W8[j,i] + F ----
                osbs = {}
                sent = set()
                chains = [(cb, ic) for cb in range(CB) for ic in range(2)]
                for g0 in range(0, len(chains), 8):
                    grp = chains[g0:g0 + 8]
                    pss = [psC.tile([P, HB], F32, tag="oc",
                                    name=f"oc{g0 + i}")
                           for i in range(len(grp))]
                    for s in range(4):
                        for (cb, ic), ps in zip(grp, pss):
                            nc.tensor.matmul(
                                ps,
                                lhsT=fs8_sb[:, 2 * s:2 * s + 2,
                                            cb * P:(cb + 1) * P],
                                rhs=W8_sb[:, 2 * s:2 * s + 2,
                                          ic * HB:(ic + 1) * HB],
                                start=(s == 0), stop=(s == 3),
                                perf_mode=mybir.MatmulPerfMode.DoubleRow,
                            )
                    for (cb, ic), ps in zip(grp, pss):
                        if cb not in osbs:
                            osbs[cb] = posb.tile([P, N], F16, tag="osb",
                                                 name=f"osb{cb}")
                        osb = osbs[cb]
                        if ic == 0:
                            nc.scalar.activation(
                                out=osb[:, :HB], in_=ps, func=AF.Identity,
                                bias=f14_sb[:, cb:cb + 1],
                            )
                        else:
                            nc.vector.tensor_scalar(
                                out=osb[:, HB:], in0=ps,
                                scalar1=f14_sb[:, cb:cb + 1],
                                scalar2=None, op0=AO.add,
                            )
                    evacd = chains[:g0 + len(grp)]
                    for cb in range(CB):
                        if cb in sent:
                            continue
                        if (cb, 0) in evacd and (cb, 1) in evacd:
                            dma_eng = nc.sync if cb % 2 == 0 else nc.scalar
                            dma_eng.dma_start(
                                out=outT[cb * P:(cb + 1) * P, :],
                                in_=osbs[cb],
                            )
                            sent.add(cb)
    nc.compile()
    return nc


_NC = None


def _get_nc():
    global _NC
    if _NC is None:
        _NC = _build_nc()
    return _NC


# ------------------------------------------------------------------- host ---

def _skew(t):
    z = np.zeros_like(t[:, 0])
    return np.stack([
        np.stack([z, -t[:, 2], t[:, 1]], -1),
        np.stack([t[:, 2], z, -t[:, 0]], -1),
        np.stack([-t[:, 1], t[:, 0], z], -1),
    ], 1)


def _fundamental(K1, K2, R, t):
    E = _skew(t) @ R
    U, S, Vt = np.linalg.svd(E)
    S = S.copy()
    S[:, 2] = 0.0
    E = U @ (S[:, :, None] * Vt)
    return np.linalg.inv(np.swapaxes(K2, 1, 2)) @ E @ np.linalg.inv(K1)


def _split3(v):
    """Triple bf16 split: v ~= hi + mid + lo (24 mantissa bits)."""
    v = v.astype(np.float32)
    hi = v.astype(BFNP)
    r1 = v - hi.astype(np.float32)
    mid = r1.astype(BFNP)
    r2 = r1 - mid.astype(np.float32)
    lo = r2.astype(BFNP)
    return hi, mid, lo


def _host_prep(f_src, K1, K2, R, t):
    ix, iy = np.meshgrid(np.arange(H, dtype=np.float32),
                         np.arange(W, dtype=np.float32), indexing="ij")
    x = ix.ravel()
    y = iy.ravel()
    comb = np.stack([x, y, np.ones(N, np.float32)], 0)  # (3,N)

    F = _fundamental(K1, K2, R, t)                    # (B,3,3)
    lines = (F @ comb).astype(np.float32)             # (B,3,N)
    lines = lines / lines[:, 2:3, :]
    y0 = -lines[:, 2, :] / lines[:, 1, :]
    y1 = -(lines[:, 2, :] + lines[:, 0, :] * np.float32(W)) / lines[:, 1, :]
    dy = y0 - y1
    L = np.sqrt(np.float32(W * W) + dy * dy)
    A5 = np.float32(5.0) * (dy / L)
    B5 = np.float32(5.0) * (np.float32(W) / L)
    C5 = np.float32(-5.0) * (np.float32(W) * y0 / L)

    Ah, Am, Al = _split3(A5)
    Bh, Bm, Bl = _split3(B5)
    Ch, Cm, Cl = _split3(C5)
    abc9 = np.stack([Ah, Bh, Ch, Am, Bm, Cm, Al, Bl, Cl], axis=1)  # (B,9,N)
    xy9 = np.tile(comb, (3, 1)).astype(BFNP)                        # (9,N)

    # Lipschitz bound on the row max: |grad d5| = 5 exactly, so
    # Mhat_i = max_j d5(coarse pt) + 5*dist is within [m_i, m_i + 56.6].
    gx = np.array([4., 12., 20., 28.], np.float32)
    cgx, cgy = np.meshgrid(gx, gx, indexing="ij")
    cgx = cgx.ravel()[:, None]
    cgy = cgy.ravel()[:, None]                                       # (16,1)
    dc = np.abs(A5[:, None, :] * cgx[None] + B5[:, None, :] * cgy[None]
                + C5[:, None, :])                                    # (B,16,N)
    mc = dc.max(-1)                                                  # (B,16)
    d2 = (x[None, :] - cgx) ** 2 + (y[None, :] - cgy) ** 2           # (16,N)
    near = np.argmin(d2, axis=0)                                     # (N,)
    dist = np.sqrt(d2[near, np.arange(N)])
    Mhat = mc[:, near] + np.float32(5.0) * dist[None, :]             # (B,N)

    fs = f_src.reshape(B, C, N).transpose(0, 2, 1)                   # (B,N,C)
    fs8 = np.clip(fs, -240, 240).astype(F8NP)
    Fcol = fs.astype(np.float64).sum(axis=1) * (SCALE / N)           # (B,C)
    f14 = Fcol.astype(np.float32).reshape(B, CB, P).transpose(0, 2, 1)  # (B,128,9)
    return abc9, xy9, Mhat, f14.astype(np.float32), fs8


def host_prep_all(f_src, K1, K2, R, t):
    abc9, xy9, Mhat, f14, fs8 = _host_prep(f_src, K1, K2, R, t)
    in_maps = [
        {"abc9": np.ascontiguousarray(abc9[b]), "xy9": xy9,
         "mhatB": np.ascontiguousarray(
             np.broadcast_to(Mhat[b], (P, N)).astype(np.float32)),
         "f14c": np.ascontiguousarray(f14[b]),
         "fs8": np.ascontiguousarray(fs8[b])}
        for b in range(B)
    ]
    return in_maps


def finish(outT_list):
    outs = np.stack([o.astype(np.float32).T for o in outT_list], 0)  # (B,N,C)
    outs *= np.float32(1.0 / SCALE)
    return outs.reshape(B, C, H, W)


def kernel(f_tar=None, f_src=None, K1=None, K2=None, R=None, t=None):
    global LAST_RESULTS
    f_src = np.asarray(f_src, np.float32)
    K1 = np.asarray(K1, np.float32)
    K2 = np.asarray(K2, np.float32)
    R = np.asarray(R, np.float32)
    t = np.asarray(t, np.float32)

    in_maps = host_prep_all(f_src, K1, K2, R, t)
    res = run_bass_kernel_spmd(_get_nc(), in_maps, list(range(B)), trace=TRACE)
    LAST_RESULTS = res
    return finish([res.results[b]["outT"] for b in range(B)])


# revision 21
# speedup vs baseline: 2.0511x; 1.0222x over previous
"""Epipolar attention kernel for Trainium2 (8 NeuronCores, batch-parallel).

Host does O(B*3^3) geometry + O(N)/O(N*C) input prep (coefficient splits,
fp8 quantization of f_src, colsum, Lipschitz row-max bound Mhat broadcast);
the device does all O(N^2) / O(N^2*C) work in a single j-major orientation:

  pass 1 (j-rows on partitions):
      dT[j,i]  = A_j x_i + B_j y_i + C_j       (PE, exact triple-bf16 split)
      xT'      = |dT| - Mhat_i                 (fused custom DVE op, PSUM in)
      u'       = exp(xT')                      (ACT)     == softmax numerator
      rhat_i   = sum_j u'[j,i]                 (PE ones-matmul column sum)
  rinv broadcast: rhat -> (1,N) -> PE one-row broadcast -> fast-reciprocal
  pass 2:
      u        = u' * rinv_i                   (DVE / GpSimd)  == exact softmax
      E2T      = exp(-u), S_j = sum_i E2T      (ACT + accum)
      W8       = E2T*(2^14/S_j) - 16           (DVE -> fp8e4) [attn = 1/N + W/2^14]
  stage 3 (c-blocks on partitions, outT = fs^T @ W^T):
      psum[c,i] = sum_j fs8[j,c] * W8[j,i]     (PE, fp8 DoubleRow, K=256/mm)
      outT = f16(psum + 2^14*F_c/N)            (ACT Identity / DVE, + bias)
Host: out = outT.T * 2^-14. The double softmax identity:
  softmax_i(1 - softmax_j(5(d-thre))) == E2/colsum(E2).
"""

import numpy as np
import ml_dtypes

import concourse.bass as bass
import concourse.bacc as bacc
import concourse.tile as tile
from concourse import mybir
from concourse.bass_utils import run_bass_kernel_spmd

# --- custom DVE op: out = |in0| - in1 (fused abs+subtract, PSUM-in) -------
from concourse import dve_ops as _dvo
from concourse.dve_ops import RECIPROCAL_APPROX_FAST, RECIP_APPROX_FAST_CONSTS
from concourse.dve_spec import Spec as _Spec, Src0 as _Src0, Src1 as _Src1, \
    Zero as _Zero, maxx as _maxx, lower as _dve_lower
from concourse.dve_uop import DveOpSpec as _DveOpSpec

_ABS_SUB_NAME = "ABS_SUB_EPI"


def _register_abs_sub():
    if _ABS_SUB_NAME in _dvo._SUB_OPCODE_FOR_NAME:
        return next(op for op in _dvo.OPS if op.name == _ABS_SUB_NAME)
    spec = _Spec(
        body=_maxx(_Src0, _Zero - _Src0) - _Src1,
        reference=lambda in0, in1, s0, s1, imm2:
            np.abs(in0.astype(np.float32)) - in1,
    )
    _dvo._SUB_OPCODE_FOR_NAME[_ABS_SUB_NAME] = (
        max(_dvo._SUB_OPCODE_FOR_NAME.values()) + 1)
    shas = {}
    for ver in ("v3", "v4"):
        s = _DveOpSpec(name=_ABS_SUB_NAME,
                       opcode=_dvo.get_dve_sub_opcode(_ABS_SUB_NAME),
                       uops=_dve_lower(spec, ver=ver), rd1_en=True)
        shas[ver] = s.sha(ver)
    op = _dvo.DveOp(_ABS_SUB_NAME, spec, subdim=False, uops_sha=shas)
    _dvo.OPS.append(op)
    _dvo.CUSTOM_DVE_SPECS[_ABS_SUB_NAME] = spec
    return op


_ABS_SUB = _register_abs_sub()

B, C, H, W = 8, 1152, 32, 32
N = H * W           # 1024
P = 128
NT = N // P         # 8
CB = C // P         # 9
F32 = mybir.dt.float32
F16 = mybir.dt.float16
BF16 = mybir.dt.bfloat16
FP8 = mybir.dt.float8e4
I32 = mybir.dt.int32
BFNP = ml_dtypes.bfloat16
F8NP = ml_dtypes.float8_e4m3

SCALE = 16384.0     # 2^14 scaling of the centered attention W
HB = 512            # psum-bank-sized half width

TRACE = False
LAST_RESULTS = None


# ----------------------------------------------------------------- device ---

def _build_nc():
    nc = bacc.Bacc()
    abc9 = nc.dram_tensor("abc9", (9, N), BF16, kind="ExternalInput")
    xy9 = nc.dram_tensor("xy9", (9, N), BF16, kind="ExternalInput")
    mhatB = nc.dram_tensor("mhatB", (P, N), F32, kind="ExternalInput")
    f14c = nc.dram_tensor("f14c", (P, CB), F32, kind="ExternalInput")
    fs8 = nc.dram_tensor("fs8", (N, C), FP8, kind="ExternalInput")
    outT = nc.dram_tensor("outT", (C, N), F16, kind="ExternalOutput")

    AF = mybir.ActivationFunctionType
    AO = mybir.AluOpType

    with tile.TileContext(nc) as tc:
        with (
            tc.tile_pool(name="consts", bufs=1) as consts,
            tc.tile_pool(name="persist", bufs=1) as persist,
            tc.tile_pool(name="pxT", bufs=3) as pxT,
            tc.tile_pool(name="pum", bufs=3) as pum,
            tc.tile_pool(name="pu", bufs=3) as pu,
            tc.tile_pool(name="pe2", bufs=6) as pe2,
            tc.tile_pool(name="posb", bufs=3) as posb,
            tc.tile_pool(name="stats", bufs=8) as stats,
        ):
            # ---- consts / inputs ----
            xy_sb = consts.tile([9, N], BF16, tag="xy")
            nc.sync.dma_start(out=xy_sb, in_=xy9[:, :])
            MhB_sb = consts.tile([P, N], F32, tag="MhB")
            nc.sync.dma_start(out=MhB_sb[:, :HB], in_=mhatB[:, :HB])
            nc.sync.dma_start(out=MhB_sb[:, HB:], in_=mhatB[:, HB:])
            abc_sb = consts.tile([9, N], BF16, tag="abc")
            nc.scalar.dma_start(out=abc_sb[:, :HB], in_=abc9[:, :HB])
            nc.scalar.dma_start(out=abc_sb[:, HB:], in_=abc9[:, HB:])
            f14_sb = consts.tile([P, CB], F32, tag="f14")
            nc.scalar.dma_start(out=f14_sb, in_=f14c[:, :])

            fs8_sb = persist.tile([P, NT, C], FP8, tag="fs8")
            for s in range(NT):
                nc.sync.dma_start(out=fs8_sb[:, s, :], in_=fs8[s * P:(s + 1) * P, :])

            W8_sb = persist.tile([P, NT, N], FP8, tag="W8")
            up_sb = persist.tile([P, NT, N], BF16, tag="up")    # u' storage
            RinvB = persist.tile([P, N], BF16, tag="RinvB")
            onesK = persist.tile([P, P], BF16, tag="onesK")     # colsum+bcast
            nc.gpsimd.memset(onesK, 1.0)

            # preload the Exp ACT table before inputs land
            dummy = stats.tile([1, 2], F32, tag="dummy")
            nc.gpsimd.memset(dummy, 0.0)
            dummy2 = stats.tile([1, 2], F32, tag="dummy2")
            nc.scalar.activation(out=dummy2, in_=dummy, func=AF.Exp)

            xts = [None] * NT

            # ---- pass 1: dT -> |dT|-Mhat -> u' = exp(.) -> rhat colsum ----
            with (
                tc.tile_pool(name="P1", bufs=2, space="PSUM") as P1,
                tc.tile_pool(name="prt", bufs=1, space="PSUM") as prt,
            ):
                # rhat colsum fused with the partition broadcast: ones
                # (128,128) lhsT makes every output partition the column sum
                rT = [prt.tile([P, HB], F32, tag=f"rt{h}", name=f"rT{h}")
                      for h in range(2)]

                def emit_dT(u, h):
                    psd = P1.tile([P, HB], F32, tag="psd", name=f"ps_{u}_{h}")
                    nc.tensor.matmul(
                        psd,
                        lhsT=abc_sb[:, u * P:(u + 1) * P],
                        rhs=xy_sb[:, h * HB:(h + 1) * HB],
                        start=True, stop=True,
                    )
                    return psd

                def emit_colsum(u, h):
                    nc.tensor.matmul(
                        rT[h],
                        lhsT=onesK,
                        rhs=up_sb[:, u, h * HB:(h + 1) * HB],
                        start=(u == 0), stop=(u == NT - 1),
                    )

                for u in range(NT):
                    xts[u] = pxT.tile([P, N], BF16, tag="xT", name=f"xT{u}")
                    for h in range(2):
                        psd = emit_dT(u, h)
                        sl = slice(h * HB, (h + 1) * HB)
                        nc.vector._custom_dve(
                            _ABS_SUB, out=xts[u][:, sl], in0=psd,
                            in1=MhB_sb[:, sl])
                    nc.scalar.activation(out=up_sb[:, u, :], in_=xts[u],
                                         func=AF.Exp)
                    # colsums trail by 2 stripes so the PE queue never waits
                    # on ACT in front of a ready d-matmul
                    if u >= 2:
                        emit_colsum(u - 2, 0)
                        emit_colsum(u - 2, 1)
                for u in (NT - 2, NT - 1):
                    emit_colsum(u, 0)
                    emit_colsum(u, 1)

                # rT already holds rhat broadcast to all partitions
                for h in range(2):
                    nc.vector._custom_dve(
                        RECIPROCAL_APPROX_FAST,
                        out=RinvB[:, h * HB:(h + 1) * HB], in0=rT[h],
                        **RECIP_APPROX_FAST_CONSTS)

            # ---- pass 2 + stage 3 (PSUM banks reclaimed for accumulators) --
            with tc.tile_pool(name="psC", bufs=8, space="PSUM") as psC:
                e2s = [None] * NT
                invs14 = [None] * NT
                pend = [None] * NT

                def emit_mult(u, eng):
                    um = pum.tile([P, N], BF16, tag="um", name=f"um{u}")
                    eng.tensor_tensor(out=um, in0=up_sb[:, u, :], in1=RinvB,
                                      op=AO.mult)
                    return um

                def emit_e2(u, um):
                    e2s[u] = pe2.tile([P, N], F16, tag="E2T", name=f"E2T{u}")
                    S1 = stats.tile([P, 1], F32, tag="S1", name=f"S1{u}")
                    nc.scalar.activation(
                        out=e2s[u], in_=um, func=AF.Exp, scale=-1.0,
                        accum_out=S1,
                    )
                    return S1

                def emit_inv(u):
                    inv = stats.tile([P, 1], F32, tag="invS", name=f"invS{u}")
                    nc.vector.reciprocal(inv, pend[u])
                    invs14[u] = stats.tile([P, 1], F32, tag="invS14",
                                           name=f"iS14{u}")
                    nc.vector.tensor_scalar_mul(invs14[u], inv, SCALE)

                def emit_w8(u):
                    nc.vector.tensor_scalar(
                        out=W8_sb[:, u, :], in0=e2s[u], scalar1=invs14[u],
                        scalar2=SCALE / N, op0=AO.mult, op1=AO.subtract,
                    )

                # everything elementwise on DVE — gpsimd's big tensor ops
                # stall concurrent DVE instructions (observed on hw)
                for u in range(NT):
                    um = emit_mult(u, nc.vector)
                    pend[u] = emit_e2(u, um)
                    if u >= 1:
                        emit_inv(u - 1)
                        emit_w8(u - 1)
                emit_inv(NT - 1)
                emit_w8(NT - 1)

                # ---- stage 3: outT[c,i] = sum_j fs8[j,c]*W8[j,i] + F ----
                osbs = {}
                sent = set()
                chains = [(cb, ic) for cb in range(CB) for ic in range(2)]
                for g0 in range(0, len(chains), 8):
                    grp = chains[g0:g0 + 8]
                    pss = [psC.tile([P, HB], F32, tag="oc",
                                    name=f"oc{g0 + i}")
                           for i in range(len(grp))]
                    for s in range(4):
                        for (cb, ic), ps in zip(grp, pss):
                            nc.tensor.matmul(
                                ps,
                                lhsT=fs8_sb[:, 2 * s:2 * s + 2,
                                            cb * P:(cb + 1) * P],
                                rhs=W8_sb[:, 2 * s:2 * s + 2,
                                          ic * HB:(ic + 1) * HB],
                                start=(s == 0), stop=(s == 3),
                                perf_mode=mybir.MatmulPerfMode.DoubleRow,
                            )
                    for (cb, ic), ps in zip(grp, pss):
                        if cb not in osbs:
                            osbs[cb] = posb.tile([P, N], F16, tag="osb",
                                                 name=f"osb{cb}")
                        osb = osbs[cb]
                        if ic == 0:
                            nc.scalar.activation(
                                out=osb[:, :HB], in_=ps, func=AF.Identity,
                                bias=f14_sb[:, cb:cb + 1],
                            )
                        else:
                            nc.vector.tensor_scalar(
                                out=osb[:, HB:], in0=ps,
                                scalar1=f14_sb[:, cb:cb + 1],
                                scalar2=None, op0=AO.add,
                            )
                    evacd = chains[:g0 + len(grp)]
                    for cb in range(CB):
                        if cb in sent:
                            continue
                        if (cb, 0) in evacd and (cb, 1) in evacd:
                            dma_eng = nc.sync if cb % 2 == 0 else nc.scalar
                            dma_eng.dma_start(
                                out=outT[cb * P:(cb + 1) * P, :],
                                in_=osbs[cb],
                            )
                            sent.add(cb)
    nc.compile()
    return nc


_NC = None


def _get_nc():
    global _NC
    if _NC is None:
        _NC = _build_nc()
    return _NC


# ------------------------------------------------------------------- host ---

def _skew(t):
    z = np.zeros_like(t[:, 0])
    return np.stack([
        np.stack([z, -t[:, 2], t[:, 1]], -1),
        np.stack([t[:, 2], z, -t[:, 0]], -1),
        np.stack([-t[:, 1], t[:, 0], z], -1),
    ], 1)


def _fundamental(K1, K2, R, t):
    E = _skew(t) @ R
    U, S, Vt = np.linalg.svd(E)
    S = S.copy()
    S[:, 2] = 0.0
    E = U @ (S[:, :, None] * Vt)
    return np.linalg.inv(np.swapaxes(K2, 1, 2)) @ E @ np.linalg.inv(K1)


def _split3(v):
    """Triple bf16 split: v ~= hi + mid + lo (24 mantissa bits)."""
    v = v.astype(np.float32)
    hi = v.astype(BFNP)
    r1 = v - hi.astype(np.float32)
    mid = r1.astype(BFNP)
    r2 = r1 - mid.astype(np.float32)
    lo = r2.astype(BFNP)
    return hi, mid, lo


def _host_prep(f_src, K1, K2, R, t):
    ix, iy = np.meshgrid(np.arange(H, dtype=np.float32),
                         np.arange(W, dtype=np.float32), indexing="ij")
    x = ix.ravel()
    y = iy.ravel()
    comb = np.stack([x, y, np.ones(N, np.float32)], 0)  # (3,N)

    F = _fundamental(K1, K2, R, t)                    # (B,3,3)
    lines = (F @ comb).astype(np.float32)             # (B,3,N)
    lines = lines / lines[:, 2:3, :]
    y0 = -lines[:, 2, :] / lines[:, 1, :]
    y1 = -(lines[:, 2, :] + lines[:, 0, :] * np.float32(W)) / lines[:, 1, :]
    dy = y0 - y1
    L = np.sqrt(np.float32(W * W) + dy * dy)
    A5 = np.float32(5.0) * (dy / L)
    B5 = np.float32(5.0) * (np.float32(W) / L)
    C5 = np.float32(-5.0) * (np.float32(W) * y0 / L)

    Ah, Am, Al = _split3(A5)
    Bh, Bm, Bl = _split3(B5)
    Ch, Cm, Cl = _split3(C5)
    abc9 = np.stack([Ah, Bh, Ch, Am, Bm, Cm, Al, Bl, Cl], axis=1)  # (B,9,N)
    xy9 = np.tile(comb, (3, 1)).astype(BFNP)                        # (9,N)

    # Lipschitz bound on the row max: |grad d5| = 5 exactly, so
    # Mhat_i = max_j d5(coarse pt) + 5*dist is within [m_i, m_i + 56.6].
    gx = np.array([4., 12., 20., 28.], np.float32)
    cgx, cgy = np.meshgrid(gx, gx, indexing="ij")
    cgx = cgx.ravel()[:, None]
    cgy = cgy.ravel()[:, None]                                       # (16,1)
    dc = np.abs(A5[:, None, :] * cgx[None] + B5[:, None, :] * cgy[None]
                + C5[:, None, :])                                    # (B,16,N)
    mc = dc.max(-1)                                                  # (B,16)
    d2 = (x[None, :] - cgx) ** 2 + (y[None, :] - cgy) ** 2           # (16,N)
    near = np.argmin(d2, axis=0)                                     # (N,)
    dist = np.sqrt(d2[near, np.arange(N)])
    Mhat = mc[:, near] + np.float32(5.0) * dist[None, :]             # (B,N)

    fs = f_src.reshape(B, C, N).transpose(0, 2, 1)                   # (B,N,C)
    fs8 = np.clip(fs, -240, 240).astype(F8NP)
    Fcol = fs.astype(np.float64).sum(axis=1) * (SCALE / N)           # (B,C)
    f14 = Fcol.astype(np.float32).reshape(B, CB, P).transpose(0, 2, 1)  # (B,128,9)
    return abc9, xy9, Mhat, f14.astype(np.float32), fs8


def host_prep_all(f_src, K1, K2, R, t):
    abc9, xy9, Mhat, f14, fs8 = _host_prep(f_src, K1, K2, R, t)
    in_maps = [
        {"abc9": np.ascontiguousarray(abc9[b]), "xy9": xy9,
         "mhatB": np.ascontiguousarray(
             np.broadcast_to(Mhat[b], (P, N)).astype(np.float32)),
         "f14c": np.ascontiguousarray(f14[b]),
         "fs8": np.ascontiguousarray(fs8[b])}
        for b in range(B)
    ]
    return in_maps


def finish(outT_list):
    outs = np.stack([o.astype(np.float32).T for o in outT_list], 0)  # (B,N,C)
    outs *= np.float32(1.0 / SCALE)
    return outs.reshape(B, C, H, W)


def kernel(f_tar=None, f_src=None, K1=None, K2=None, R=None, t=None):
    global LAST_RESULTS
    f_src = np.asarray(f_src, np.float32)
    K1 = np.asarray(K1, np.float32)
    K2 = np.asarray(K2, np.float32)
    R = np.asarray(R, np.float32)
    t = np.asarray(t, np.float32)

    in_maps = host_prep_all(f_src, K1, K2, R, t)
    res = run_bass_kernel_spmd(_get_nc(), in_maps, list(range(B)), trace=TRACE)
    LAST_RESULTS = res
    return finish([res.results[b]["outT"] for b in range(B)])


# revision 24
# speedup vs baseline: 2.0878x; 1.0179x over previous
"""Epipolar attention kernel for Trainium2 (8 NeuronCores, batch-parallel).

Host does O(B*3^3) geometry + O(N)/O(N*C) input prep (coefficient splits,
fp8 quantization of f_src, colsum, Lipschitz row-max bound Mhat broadcast);
the device does all O(N^2) / O(N^2*C) work in a single j-major orientation:

  pass 1 (j-rows on partitions):
      dT[j,i]  = A_j x_i + B_j y_i + C_j       (PE, exact triple-bf16 split)
      xT'      = |dT| - Mhat_i                 (fused custom DVE op, PSUM in)
      u'       = exp(xT')                      (ACT)     == softmax numerator
      rhat_i   = sum_j u'[j,i]                 (PE ones-matmul column sum)
  rinv broadcast: rhat -> (1,N) -> PE one-row broadcast -> fast-reciprocal
  pass 2:
      u        = u' * rinv_i                   (DVE / GpSimd)  == exact softmax
      E2T      = exp(-u), S_j = sum_i E2T      (ACT + accum)
      W8       = E2T*(2^14/S_j) - 16           (DVE -> fp8e4) [attn = 1/N + W/2^14]
  stage 3 (c-blocks on partitions, outT = fs^T @ W^T):
      psum[c,i] = sum_j fs8[j,c] * W8[j,i]     (PE, fp8 DoubleRow, K=256/mm)
      outT = f16(psum + 2^14*F_c/N)            (ACT Identity / DVE, + bias)
Host: out = outT.T * 2^-14. The double softmax identity:
  softmax_i(1 - softmax_j(5(d-thre))) == E2/colsum(E2).
"""

import numpy as np
import ml_dtypes

import concourse.bass as bass
import concourse.bacc as bacc
import concourse.tile as tile
from concourse import mybir
from concourse.bass_utils import run_bass_kernel_spmd

# --- custom DVE op: out = |in0| - in1 (fused abs+subtract, PSUM-in) -------
from concourse import dve_ops as _dvo
from concourse.dve_ops import RECIPROCAL_APPROX_FAST, RECIP_APPROX_FAST_CONSTS
from concourse.dve_spec import Spec as _Spec, Src0 as _Src0, Src1 as _Src1, \
    Zero as _Zero, maxx as _maxx, lower as _dve_lower
from concourse.dve_uop import DveOpSpec as _DveOpSpec

_ABS_SUB_NAME = "ABS_SUB_EPI"


def _register_abs_sub():
    if _ABS_SUB_NAME in _dvo._SUB_OPCODE_FOR_NAME:
        return next(op for op in _dvo.OPS if op.name == _ABS_SUB_NAME)
    spec = _Spec(
        body=_maxx(_Src0, _Zero - _Src0) - _Src1,
        reference=lambda in0, in1, s0, s1, imm2:
            np.abs(in0.astype(np.float32)) - in1,
    )
    _dvo._SUB_OPCODE_FOR_NAME[_ABS_SUB_NAME] = (
        max(_dvo._SUB_OPCODE_FOR_NAME.values()) + 1)
    shas = {}
    for ver in ("v3", "v4"):
        s = _DveOpSpec(name=_ABS_SUB_NAME,
                       opcode=_dvo.get_dve_sub_opcode(_ABS_SUB_NAME),
                       uops=_dve_lower(spec, ver=ver), rd1_en=True)
        shas[ver] = s.sha(ver)
    op = _dvo.DveOp(_ABS_SUB_NAME, spec, subdim=False, uops_sha=shas)
    _dvo.OPS.append(op)
    _dvo.CUSTOM_DVE_SPECS[_ABS_SUB_NAME] = spec
    return op


_ABS_SUB = _register_abs_sub()

B, C, H, W = 8, 1152, 32, 32
N = H * W           # 1024
P = 128
NT = N // P         # 8
CB = C // P         # 9
F32 = mybir.dt.float32
F16 = mybir.dt.float16
BF16 = mybir.dt.bfloat16
FP8 = mybir.dt.float8e4
I32 = mybir.dt.int32
BFNP = ml_dtypes.bfloat16
F8NP = ml_dtypes.float8_e4m3

SCALE = 16384.0     # 2^14 scaling of the centered attention W
HB = 512            # psum-bank-sized half width

TRACE = False
LAST_RESULTS = None


# ----------------------------------------------------------------- device ---

def _build_nc():
    nc = bacc.Bacc()
    abc9 = nc.dram_tensor("abc9", (9, N), BF16, kind="ExternalInput")
    xy9 = nc.dram_tensor("xy9", (9, N), BF16, kind="ExternalInput")
    mhatB = nc.dram_tensor("mhatB", (P, N), F32, kind="ExternalInput")
    f14c = nc.dram_tensor("f14c", (P, CB), F32, kind="ExternalInput")
    fs8 = nc.dram_tensor("fs8", (N, C), FP8, kind="ExternalInput")
    outT = nc.dram_tensor("outT", (C, N), F16, kind="ExternalOutput")

    AF = mybir.ActivationFunctionType
    AO = mybir.AluOpType

    with tile.TileContext(nc) as tc:
        with (
            tc.tile_pool(name="consts", bufs=1) as consts,
            tc.tile_pool(name="persist", bufs=1) as persist,
            tc.tile_pool(name="pxT", bufs=3) as pxT,
            tc.tile_pool(name="pum", bufs=3) as pum,
            tc.tile_pool(name="pu", bufs=3) as pu,
            tc.tile_pool(name="pe2", bufs=6) as pe2,
            tc.tile_pool(name="posb", bufs=3) as posb,
            tc.tile_pool(name="stats", bufs=8) as stats,
        ):
            # ---- consts / inputs ----
            xy_sb = consts.tile([9, N], BF16, tag="xy")
            nc.sync.dma_start(out=xy_sb, in_=xy9[:, :])
            MhB_sb = consts.tile([P, N], F32, tag="MhB")
            nc.sync.dma_start(out=MhB_sb[:, :HB], in_=mhatB[:, :HB])
            nc.sync.dma_start(out=MhB_sb[:, HB:], in_=mhatB[:, HB:])
            abc_sb = consts.tile([9, N], BF16, tag="abc")
            nc.scalar.dma_start(out=abc_sb[:, :HB], in_=abc9[:, :HB])
            nc.scalar.dma_start(out=abc_sb[:, HB:], in_=abc9[:, HB:])
            f14_sb = consts.tile([P, CB], F32, tag="f14")
            nc.scalar.dma_start(out=f14_sb, in_=f14c[:, :])

            fs8_sb = persist.tile([P, NT, C], FP8, tag="fs8")
            for s in range(NT):
                nc.sync.dma_start(out=fs8_sb[:, s, :], in_=fs8[s * P:(s + 1) * P, :])

            W8_sb = persist.tile([P, NT, N], FP8, tag="W8")
            up_sb = persist.tile([P, NT, N], BF16, tag="up")    # u' storage
            RinvB = persist.tile([P, N], BF16, tag="RinvB")
            onesK = persist.tile([P, P], BF16, tag="onesK")     # colsum+bcast
            nc.gpsimd.memset(onesK, 1.0)

            # preload the Exp ACT table before inputs land
            dummy = stats.tile([1, 2], F32, tag="dummy")
            nc.gpsimd.memset(dummy, 0.0)
            dummy2 = stats.tile([1, 2], F32, tag="dummy2")
            nc.scalar.activation(out=dummy2, in_=dummy, func=AF.Exp)

            xts = [None] * NT

            # ---- pass 1: dT -> |dT|-Mhat -> u' = exp(.) -> rhat colsum ----
            with (
                tc.tile_pool(name="P1", bufs=2, space="PSUM") as P1,
                tc.tile_pool(name="prt", bufs=1, space="PSUM") as prt,
            ):
                # rhat colsum fused with the partition broadcast: ones
                # (128,128) lhsT makes every output partition the column sum
                rT = [prt.tile([P, HB], F32, tag=f"rt{h}", name=f"rT{h}")
                      for h in range(2)]

                def emit_dT(u, h):
                    psd = P1.tile([P, HB], F32, tag="psd", name=f"ps_{u}_{h}")
                    nc.tensor.matmul(
                        psd,
                        lhsT=abc_sb[:, u * P:(u + 1) * P],
                        rhs=xy_sb[:, h * HB:(h + 1) * HB],
                        start=True, stop=True,
                    )
                    return psd

                def emit_colsum(rhs_full, h, start, stop):
                    nc.tensor.matmul(
                        rT[h],
                        lhsT=onesK,
                        rhs=rhs_full[:, h * HB:(h + 1) * HB],
                        start=start, stop=stop,
                    )

                # stripe pairs 0-5 are pre-summed on the idle gpsimd so the
                # PE only runs 10 colsum matmuls instead of 16; stripes 6,7
                # are summed directly so no gp latency lands on the tail
                usums = []
                for u in range(NT):
                    xts[u] = pxT.tile([P, N], BF16, tag="xT", name=f"xT{u}")
                    for h in range(2):
                        psd = emit_dT(u, h)
                        sl = slice(h * HB, (h + 1) * HB)
                        nc.vector._custom_dve(
                            _ABS_SUB, out=xts[u][:, sl], in0=psd,
                            in1=MhB_sb[:, sl])
                    nc.scalar.activation(out=up_sb[:, u, :], in_=xts[u],
                                         func=AF.Exp)
                    if u in (1, 3):
                        us = pum.tile([P, N], BF16, tag="um",
                                      name=f"usum{u // 2}")
                        nc.gpsimd.tensor_tensor(
                            out=us, in0=up_sb[:, u - 1, :],
                            in1=up_sb[:, u, :], op=AO.add)
                        usums.append(us)
                    # colsums trail so the PE queue never waits on ACT/gp in
                    # front of a ready d-matmul
                    if u in (3, 5):
                        k = u // 2 - 1
                        for h in range(2):
                            emit_colsum(usums[k], h, start=(k == 0),
                                        stop=False)
                    if u in (6, 7):
                        for h in range(2):
                            emit_colsum(up_sb[:, u - 2, :], h, start=False,
                                        stop=False)
                for u in (NT - 2, NT - 1):
                    for h in range(2):
                        emit_colsum(up_sb[:, u, :], h, start=False,
                                    stop=(u == NT - 1))

                # rT already holds rhat broadcast to all partitions
                for h in range(2):
                    nc.vector._custom_dve(
                        RECIPROCAL_APPROX_FAST,
                        out=RinvB[:, h * HB:(h + 1) * HB], in0=rT[h],
                        **RECIP_APPROX_FAST_CONSTS)

            # ---- pass 2 + stage 3 (PSUM banks reclaimed for accumulators) --
            with tc.tile_pool(name="psC", bufs=8, space="PSUM") as psC:
                e2s = [None] * NT
                invs14 = [None] * NT
                pend = [None] * NT

                def emit_mult(u, eng):
                    um = pum.tile([P, N], BF16, tag="um", name=f"um{u}")
                    eng.tensor_tensor(out=um, in0=up_sb[:, u, :], in1=RinvB,
                                      op=AO.mult)
                    return um

                def emit_e2(u, um):
                    e2s[u] = pe2.tile([P, N], F16, tag="E2T", name=f"E2T{u}")
                    S1 = stats.tile([P, 1], F32, tag="S1", name=f"S1{u}")
                    nc.scalar.activation(
                        out=e2s[u], in_=um, func=AF.Exp, scale=-1.0,
                        accum_out=S1,
                    )
                    return S1

                def emit_inv(u):
                    inv = stats.tile([P, 1], F32, tag="invS", name=f"invS{u}")
                    nc.vector.reciprocal(inv, pend[u])
                    invs14[u] = stats.tile([P, 1], F32, tag="invS14",
                                           name=f"iS14{u}")
                    nc.vector.tensor_scalar_mul(invs14[u], inv, SCALE)

                def emit_w8(u):
                    nc.vector.tensor_scalar(
                        out=W8_sb[:, u, :], in0=e2s[u], scalar1=invs14[u],
                        scalar2=SCALE / N, op0=AO.mult, op1=AO.subtract,
                    )

                # everything elementwise on DVE — gpsimd's big tensor ops
                # stall concurrent DVE instructions (observed on hw)
                for u in range(NT):
                    um = emit_mult(u, nc.vector)
                    pend[u] = emit_e2(u, um)
                    if u >= 1:
                        emit_inv(u - 1)
                        emit_w8(u - 1)
                emit_inv(NT - 1)
                emit_w8(NT - 1)

                # ---- stage 3: outT[c,i] = sum_j fs8[j,c]*W8[j,i] + F ----
                osbs = {}
                sent = set()
                chains = [(cb, ic) for cb in range(CB) for ic in range(2)]
                for g0 in range(0, len(chains), 8):
                    grp = chains[g0:g0 + 8]
                    pss = [psC.tile([P, HB], F32, tag="oc",
                                    name=f"oc{g0 + i}")
                           for i in range(len(grp))]
                    for s in range(4):
                        for (cb, ic), ps in zip(grp, pss):
                            nc.tensor.matmul(
                                ps,
                                lhsT=fs8_sb[:, 2 * s:2 * s + 2,
                                            cb * P:(cb + 1) * P],
                                rhs=W8_sb[:, 2 * s:2 * s + 2,
                                          ic * HB:(ic + 1) * HB],
                                start=(s == 0), stop=(s == 3),
                                perf_mode=mybir.MatmulPerfMode.DoubleRow,
                            )
                    for (cb, ic), ps in zip(grp, pss):
                        if cb not in osbs:
                            osbs[cb] = posb.tile([P, N], F16, tag="osb",
                                                 name=f"osb{cb}")
                        osb = osbs[cb]
                        if ic == 0:
                            nc.scalar.activation(
                                out=osb[:, :HB], in_=ps, func=AF.Identity,
                                bias=f14_sb[:, cb:cb + 1],
                            )
                        else:
                            nc.vector.tensor_scalar(
                                out=osb[:, HB:], in0=ps,
                                scalar1=f14_sb[:, cb:cb + 1],
                                scalar2=None, op0=AO.add,
                            )
                    evacd = chains[:g0 + len(grp)]
                    for cb in range(CB):
                        if cb in sent:
                            continue
                        if (cb, 0) in evacd and (cb, 1) in evacd:
                            dma_eng = nc.sync if cb % 2 == 0 else nc.scalar
                            dma_eng.dma_start(
                                out=outT[cb * P:(cb + 1) * P, :],
                                in_=osbs[cb],
                            )
                            sent.add(cb)
    nc.compile()
    return nc


_NC = None


def _get_nc():
    global _NC
    if _NC is None:
        _NC = _build_nc()
    return _NC


# ------------------------------------------------------------------- host ---

def _skew(t):
    z = np.zeros_like(t[:, 0])
    return np.stack([
        np.stack([z, -t[:, 2], t[:, 1]], -1),
        np.stack([t[:, 2], z, -t[:, 0]], -1),
        np.stack([-t[:, 1], t[:, 0], z], -1),
    ], 1)


def _fundamental(K1, K2, R, t):
    E = _skew(t) @ R
    U, S, Vt = np.linalg.svd(E)
    S = S.copy()
    S[:, 2] = 0.0
    E = U @ (S[:, :, None] * Vt)
    return np.linalg.inv(np.swapaxes(K2, 1, 2)) @ E @ np.linalg.inv(K1)


def _split3(v):
    """Triple bf16 split: v ~= hi + mid + lo (24 mantissa bits)."""
    v = v.astype(np.float32)
    hi = v.astype(BFNP)
    r1 = v - hi.astype(np.float32)
    mid = r1.astype(BFNP)
    r2 = r1 - mid.astype(np.float32)
    lo = r2.astype(BFNP)
    return hi, mid, lo


def _host_prep(f_src, K1, K2, R, t):
    ix, iy = np.meshgrid(np.arange(H, dtype=np.float32),
                         np.arange(W, dtype=np.float32), indexing="ij")
    x = ix.ravel()
    y = iy.ravel()
    comb = np.stack([x, y, np.ones(N, np.float32)], 0)  # (3,N)

    F = _fundamental(K1, K2, R, t)                    # (B,3,3)
    lines = (F @ comb).astype(np.float32)             # (B,3,N)
    lines = lines / lines[:, 2:3, :]
    y0 = -lines[:, 2, :] / lines[:, 1, :]
    y1 = -(lines[:, 2, :] + lines[:, 0, :] * np.float32(W)) / lines[:, 1, :]
    dy = y0 - y1
    L = np.sqrt(np.float32(W * W) + dy * dy)
    A5 = np.float32(5.0) * (dy / L)
    B5 = np.float32(5.0) * (np.float32(W) / L)
    C5 = np.float32(-5.0) * (np.float32(W) * y0 / L)

    Ah, Am, Al = _split3(A5)
    Bh, Bm, Bl = _split3(B5)
    Ch, Cm, Cl = _split3(C5)
    abc9 = np.stack([Ah, Bh, Ch, Am, Bm, Cm, Al, Bl, Cl], axis=1)  # (B,9,N)
    xy9 = np.tile(comb, (3, 1)).astype(BFNP)                        # (9,N)

    # Lipschitz bound on the row max: |grad d5| = 5 exactly, so
    # Mhat_i = max_j d5(coarse pt) + 5*dist is within [m_i, m_i + 56.6].
    gx = np.array([4., 12., 20., 28.], np.float32)
    cgx, cgy = np.meshgrid(gx, gx, indexing="ij")
    cgx = cgx.ravel()[:, None]
    cgy = cgy.ravel()[:, None]                                       # (16,1)
    dc = np.abs(A5[:, None, :] * cgx[None] + B5[:, None, :] * cgy[None]
                + C5[:, None, :])                                    # (B,16,N)
    mc = dc.max(-1)                                                  # (B,16)
    d2 = (x[None, :] - cgx) ** 2 + (y[None, :] - cgy) ** 2           # (16,N)
    near = np.argmin(d2, axis=0)                                     # (N,)
    dist = np.sqrt(d2[near, np.arange(N)])
    Mhat = mc[:, near] + np.float32(5.0) * dist[None, :]             # (B,N)

    fs = f_src.reshape(B, C, N).transpose(0, 2, 1)                   # (B,N,C)
    fs8 = np.clip(fs, -240, 240).astype(F8NP)
    Fcol = fs.astype(np.float64).sum(axis=1) * (SCALE / N)           # (B,C)
    f14 = Fcol.astype(np.float32).reshape(B, CB, P).transpose(0, 2, 1)  # (B,128,9)
    return abc9, xy9, Mhat, f14.astype(np.float32), fs8


def host_prep_all(f_src, K1, K2, R, t):
    abc9, xy9, Mhat, f14, fs8 = _host_prep(f_src, K1, K2, R, t)
    in_maps = [
        {"abc9": np.ascontiguousarray(abc9[b]), "xy9": xy9,
         "mhatB": np.ascontiguousarray(
             np.broadcast_to(Mhat[b], (P, N)).astype(np.float32)),
         "f14c": np.ascontiguousarray(f14[b]),
         "fs8": np.ascontiguousarray(fs8[b])}
        for b in range(B)
    ]
    return in_maps


def finish(outT_list):
    outs = np.stack([o.astype(np.float32).T for o in outT_list], 0)  # (B,N,C)
    outs *= np.float32(1.0 / SCALE)
    return outs.reshape(B, C, H, W)


def kernel(f_tar=None, f_src=None, K1=None, K2=None, R=None, t=None):
    global LAST_RESULTS
    f_src = np.asarray(f_src, np.float32)
    K1 = np.asarray(K1, np.float32)
    K2 = np.asarray(K2, np.float32)
    R = np.asarray(R, np.float32)
    t = np.asarray(t, np.float32)

    in_maps = host_prep_all(f_src, K1, K2, R, t)
    res = run_bass_kernel_spmd(_get_nc(), in_maps, list(range(B)), trace=TRACE)
    LAST_RESULTS = res
    return finish([res.results[b]["outT"] for b in range(B)])
